# revision 1
# baseline (speedup 1.0000x reference)
"""GraphSAGE 2-layer forward on 8 TRN2 NeuronCores.

Strategy (graph/data parallel per sharding hint):
- Nodes dst-sharded across 8 cores (6250 nodes/core, 49 tiles of 128).
- Host sorts edges by dst, buckets per (core, dst-tile), splits by src<32768
  (dma_gather idx is int16) and pads each bucket to 128-slot chunks.
- L1: gather x_bf16[src] rows (256B) from HBM via gpsimd.dma_gather;
  scatter-add via one-hot matmuls into PSUM (one-hot built on DVE with
  iota + is_equal against per-slot dst values); mean via per-partition
  inv-degree scale; dense W1_l/W1_r matmuls (f32) fused bias+relu on ACT.
- h kept transposed [hid, nodes] in SBUF; p = h @ W2_l computed row-major,
  AllGathered (bf16, 128-col padded rows) so every core can gather p[src].
- L2: same gather/scatter machinery on p; + h @ W2_r + b2; log_softmax
  along the free dim; DMA out.
"""

import numpy as np
import ml_dtypes

import concourse.bacc as bacc
import concourse.bass as bass
import concourse.mybir as mybir
import concourse.tile as tile
from concourse.bass_utils import run_bass_kernel_spmd

N = 50000
F = 128
HID = 256
CLS = 47
CORES = 8
NPC = N // CORES           # 6250
TPC = (NPC + 127) // 128   # 49 tiles per core
SPLIT = 32768              # int16 index limit for dma_gather
GPT = 7                    # dst-tiles per gather group
NG = (TPC + GPT - 1) // GPT

f32 = mybir.dt.float32
bf16 = mybir.dt.bfloat16
i16 = mybir.dt.int16
ALU = mybir.AluOpType
ACTF = mybir.ActivationFunctionType

IOTA_BF = np.tile(np.arange(128, dtype=np.float32)[None, :],
                  (128, 1)).astype(ml_dtypes.bfloat16)
IDENT_F32 = np.eye(128, dtype=np.float32)


def _host_prep(x, edge_index):
    src = np.asarray(edge_index[0], np.int64)
    dst = np.asarray(edge_index[1], np.int64)
    deg = np.bincount(dst, minlength=N).astype(np.float32)

    order = np.argsort(dst, kind="stable")
    src_s = src[order]
    dst_s = dst[order]
    bounds = np.searchsorted(dst_s, np.arange(0, N + 1, NPC))

    seg_idx = {}
    cnt = np.zeros((CORES, TPC, 2), np.int64)
    for c in range(CORES):
        sl = slice(bounds[c], bounds[c + 1])
        sc = src_s[sl]
        dcl = dst_s[sl] - c * NPC
        tt = dcl >> 7
        t_ord = np.argsort(tt, kind="stable")
        sc, dcl, tt = sc[t_ord], dcl[t_ord], tt[t_ord]
        tb = np.searchsorted(tt, np.arange(TPC + 1))
        for t in range(TPC):
            s2 = slice(tb[t], tb[t + 1])
            s_t = sc[s2]
            d_t = dcl[s2] & 127
            lo = s_t < SPLIT
            seg_idx[(c, t, 0)] = (s_t[lo], d_t[lo])
            seg_idx[(c, t, 1)] = (s_t[~lo] - SPLIT, d_t[~lo])
            cnt[c, t, 0] = int(lo.sum())
            cnt[c, t, 1] = int((~lo).sum())

    # chunk counts, uniform across cores (SPMD single program)
    nch = np.ceil(cnt / 128.0).astype(np.int64).max(axis=0)  # [TPC, 2]

    groups = []
    chunk_ptr = 0
    for g in range(NG):
        tiles = list(range(g * GPT, min((g + 1) * GPT, TPC)))
        seg_chunks = {0: {}, 1: {}}
        base = chunk_ptr
        for s in (0, 1):
            for t in tiles:
                seg_chunks[s][t] = (chunk_ptr, int(nch[t, s]))
                chunk_ptr += int(nch[t, s])
        groups.append(dict(tiles=tiles, seg_chunks=seg_chunks, base=base,
                           nchunks=chunk_ptr - base))
    tot_ch = chunk_ptr
    W = tot_ch * 8  # idx columns: 128 slots/chunk / 16

    gidx_all, dstv_all, degp_all, xown_all = [], [], [], []
    for c in range(CORES):
        gi = np.zeros((16, W), np.int16)
        dv = np.full((128, tot_ch), -1.0, np.float32)
        for t in range(TPC):
            g = t // GPT
            for s in (0, 1):
                c0, ncks = groups[g]["seg_chunks"][s][t]
                if ncks == 0:
                    continue
                iv, dl = seg_idx[(c, t, s)]
                S = ncks * 128
                ivp = np.zeros(S, np.int64)
                ivp[: len(iv)] = iv
                dvp = np.full(S, -1.0, np.float32)
                dvp[: len(dl)] = dl
                gi[:, c0 * 8:(c0 + ncks) * 8] = ivp.reshape(-1, 16).T
                dv[:, c0:c0 + ncks] = dvp.reshape(ncks, 128).T
        gidx_all.append(np.tile(gi, (8, 1)))  # replicate across 8 Q7 cores
        dstv_all.append(dv)
        dpc = np.ones(TPC * 128, np.float32)
        dpc[:NPC] = deg[c * NPC:(c + 1) * NPC]
        degp_all.append(np.ascontiguousarray(dpc.reshape(TPC, 128).T))
        xo = np.zeros((TPC * 128, F), np.float32)
        xo[:NPC] = x[c * NPC:(c + 1) * NPC]
        xown_all.append(xo)

    sched = dict(groups=groups, tot_ch=tot_ch, W=W,
                 max_gch=max(g["nchunks"] for g in groups))
    return sched, gidx_all, dstv_all, degp_all, xown_all


def _build(sched, phases=3):
    groups, tot_ch, W = sched["groups"], sched["tot_ch"], sched["W"]
    max_gch = sched["max_gch"]

    nc = bacc.Bacc("TRN2", num_devices=CORES)
    xbf_h = nc.declare_dram_parameter("xbf", [N, F], bf16, False)
    xown_h = nc.declare_dram_parameter("xown", [TPC * 128, F], f32, False)
    gidx_h = nc.declare_dram_parameter("gidx", [128, W], i16, False)
    iotab_h = nc.declare_dram_parameter("iotab", [128, 128], bf16, False)
    ident_h = nc.declare_dram_parameter("ident", [128, 128], f32, False)
    dstv_h = nc.declare_dram_parameter("dstv", [128, tot_ch], f32, False)
    degp_h = nc.declare_dram_parameter("degp", [128, TPC], f32, False)
    w1l_h = nc.declare_dram_parameter("w1l", [F, HID], f32, False)
    w1r_h = nc.declare_dram_parameter("w1r", [F, HID], f32, False)
    w2l_h = nc.declare_dram_parameter("w2l", [128, 2 * CLS], f32, False)
    w2r_h = nc.declare_dram_parameter("w2r", [128, 2 * CLS], f32, False)
    b1_h = nc.declare_dram_parameter("b1c", [128, 2], f32, False)
    b2_h = nc.declare_dram_parameter("b2r", [1, CLS], f32, False)
    out_h = nc.declare_dram_parameter("out", [NPC, CLS], f32, True)

    p_loc = nc.dram_tensor("p_loc", [NPC, 128], bf16)
    p_full = nc.dram_tensor("p_full", [N, 128], bf16, addr_space="Shared")

    with tile.TileContext(nc) as tc:
        with (
            tc.tile_pool(name="const", bufs=1) as cp,
            tc.tile_pool(name="msg", bufs=2) as msgp,
            tc.tile_pool(name="oh", bufs=6) as ohp,
            tc.tile_pool(name="sb", bufs=3) as sbp,
            tc.tile_pool(name="small", bufs=4) as smp,
        ):
            # ---- persistent tiles ----
            idx_sb = cp.tile([128, W], i16, tag="idx")
            nc.sync.dma_start(idx_sb[:], gidx_h[:, :])
            dstv_sb = cp.tile([128, tot_ch], f32, tag="dstv")
            nc.sync.dma_start(dstv_sb[:], dstv_h[:, :])
            w1l_sb = cp.tile([F, HID], f32, tag="w1l")
            nc.sync.dma_start(w1l_sb[:], w1l_h[:, :])
            w1r_sb = cp.tile([F, HID], f32, tag="w1r")
            nc.sync.dma_start(w1r_sb[:], w1r_h[:, :])
            w2l_sb = cp.tile([128, 2 * CLS], f32, tag="w2l")
            nc.sync.dma_start(w2l_sb[:], w2l_h[:, :])
            w2r_sb = cp.tile([128, 2 * CLS], f32, tag="w2r")
            nc.sync.dma_start(w2r_sb[:], w2r_h[:, :])
            b1_sb = cp.tile([128, 2], f32, tag="b1")
            nc.sync.dma_start(b1_sb[:], b1_h[:, :])
            b2_sb = cp.tile([1, CLS], f32, tag="b2")
            nc.sync.dma_start(b2_sb[:], b2_h[:, :])
            deg_sb = cp.tile([128, TPC], f32, tag="deg")
            nc.sync.dma_start(deg_sb[:], degp_h[:, :])

            inv_sb = cp.tile([128, TPC], f32, tag="inv")
            nc.vector.tensor_scalar(inv_sb[:], deg_sb[:], 1.0, None, ALU.max)
            nc.vector.reciprocal(inv_sb[:], inv_sb[:])

            iota_bf = cp.tile([128, 128], bf16, tag="iotabf")
            nc.sync.dma_start(iota_bf[:], iotab_h[:, :])
            ident = cp.tile([128, 128], f32, tag="ident")
            nc.sync.dma_start(ident[:], ident_h[:, :])
            ones_sb = cp.tile([1, 128], f32, tag="ones")
            nc.vector.memset(ones_sb[:], 1.0)

            h1T0 = cp.tile([128, TPC * 128], f32, tag="h1a")
            h1T1 = cp.tile([128, TPC * 128], f32, tag="h1b")

            def gathers(group, table_lo, table_hi, msg3):
                """Issue lo/hi dma_gather for one group into msg3 [128,C,128]."""
                base = group["base"]
                n_lo = sum(n for (_, n) in group["seg_chunks"][0].values())
                n_hi = sum(n for (_, n) in group["seg_chunks"][1].values())
                if n_lo:
                    S = n_lo * 128
                    nc.gpsimd.dma_gather(
                        msg3[:, 0:n_lo, :], table_lo,
                        idx_sb[:, base * 8:(base + n_lo) * 8],
                        S, S, F, single_packet=False)
                if n_hi:
                    S = n_hi * 128
                    nc.gpsimd.dma_gather(
                        msg3[:, n_lo:n_lo + n_hi, :], table_hi,
                        idx_sb[:, (base + n_lo) * 8:(base + n_lo + n_hi) * 8],
                        S, S, F, single_packet=False)

            def agg_tile_chunks(group, t, msg3, psl):
                """One-hot matmuls accumulating agg for dst-tile t."""
                base = group["base"]
                lo0, nlo = group["seg_chunks"][0][t]
                hi0, nhi = group["seg_chunks"][1][t]
                gcs = [lo0 + k for k in range(nlo)] + \
                      [hi0 + k for k in range(nhi)]
                for i, gc in enumerate(gcs):
                    oh = ohp.tile([128, 128], bf16, tag="oh")
                    nc.vector.tensor_scalar(oh[:], iota_bf[:],
                                            dstv_sb[:, gc:gc + 1], None,
                                            ALU.is_equal)
                    nc.tensor.matmul(psl, oh[:], msg3[:, gc - base, :],
                                     start=(i == 0), stop=(i == len(gcs) - 1))
                return len(gcs) > 0

            # =============== Layer 1 ===============
            with (
                tc.tile_pool(name="aggps", bufs=3, space="PSUM") as aggpp,
                tc.tile_pool(name="tp", bufs=2, space="PSUM") as tpp,
                tc.tile_pool(name="zp", bufs=2, space="PSUM") as zpp,
            ):
                for g in range(NG):
                    grp = groups[g]
                    gch = grp["nchunks"]
                    msg = msgp.tile([128, max_gch * 128], bf16, tag="msg")
                    msg3 = msg[:].rearrange("p (c e) -> p c e", e=F)
                    gathers(grp, xbf_h[0:SPLIT, :], xbf_h[SPLIT:N, :], msg3)
                    for tl, t in enumerate(grp["tiles"]):
                        agg_ps = aggpp.tile([128, 128], f32, tag="agg")
                        nonempty = agg_tile_chunks(grp, t, msg3, agg_ps[:])
                        mean = sbp.tile([128, 128], f32, tag="mean")
                        if nonempty:
                            nc.vector.tensor_scalar(
                                mean[:], agg_ps[:],
                                inv_sb[:, t:t + 1], None, ALU.mult)
                        else:
                            nc.vector.memset(mean[:], 0.0)
                        mt_ps = tpp.tile([128, 128], f32, tag="tp")
                        nc.tensor.transpose(mt_ps[:], mean[:], ident[:])
                        meanT = sbp.tile([128, 128], f32, tag="meanT")
                        nc.scalar.activation(meanT[:], mt_ps[:], ACTF.Copy)
                        xo = sbp.tile([128, 128], f32, tag="xo")
                        nc.sync.dma_start(xo[:], xown_h[t * 128:(t + 1) * 128, :])
                        xt_ps = tpp.tile([128, 128], f32, tag="tp")
                        nc.tensor.transpose(xt_ps[:], xo[:], ident[:])
                        xoT = sbp.tile([128, 128], f32, tag="xoT")
                        nc.scalar.activation(xoT[:], xt_ps[:], ACTF.Copy)
                        z_ps = zpp.tile([128, 256], f32, tag="z")
                        for h, h1T in ((0, h1T0), (1, h1T1)):
                            zs = z_ps[:, h * 128:(h + 1) * 128]
                            nc.tensor.matmul(zs, w1l_sb[:, h * 128:(h + 1) * 128],
                                             meanT[:], start=True, stop=False)
                            nc.tensor.matmul(zs, w1r_sb[:, h * 128:(h + 1) * 128],
                                             xoT[:], start=False, stop=True)
                            nc.scalar.activation(h1T[:, t * 128:(t + 1) * 128],
                                                 zs, ACTF.Relu,
                                                 bias=b1_sb[:, h:h + 1],
                                                 scale=1.0)

            # =============== p = h @ W2_l, AllGather ===============
            with tc.tile_pool(name="pp", bufs=2, space="PSUM") as ppp:
                if phases < 2:
                    for t in range(TPC):
                        res = smp.tile([128, CLS], f32, tag="res")
                        nc.vector.tensor_copy(res[:], h1T0[:, t * 128:t * 128 + CLS])
                        rows = NPC - t * 128 if t == TPC - 1 else 128
                        nc.sync.dma_start(out_h[t * 128:t * 128 + rows, :], res[0:rows, :])
                for t in (range(TPC) if phases >= 2 else []):
                    ts = slice(t * 128, (t + 1) * 128)
                    pp_ps = ppp.tile([128, 64], f32, tag="pp")
                    nc.tensor.matmul(pp_ps[:, 0:CLS], h1T0[:, ts],
                                     w2l_sb[:, 0:CLS], start=True, stop=False)
                    nc.tensor.matmul(pp_ps[:, 0:CLS], h1T1[:, ts],
                                     w2l_sb[:, CLS:2 * CLS], start=False,
                                     stop=True)
                    psb = sbp.tile([128, 128], bf16, tag="psb")
                    nc.vector.memset(psb[:, CLS:128], 0.0)
                    nc.scalar.activation(psb[:, 0:CLS], pp_ps[:, 0:CLS],
                                         ACTF.Copy)
                    rows = NPC - t * 128 if t == TPC - 1 else 128
                    nc.sync.dma_start(p_loc[t * 128:t * 128 + rows, :],
                                      psb[0:rows, :])

                if phases >= 2:
                    nc.gpsimd.collective_compute(
                        "AllGather", ALU.bypass,
                        replica_groups=[list(range(CORES))],
                        ins=[p_loc.ap().opt()], outs=[p_full.ap().opt()])

                # b2 broadcast across partitions via rank-1 matmul
                b2_ps = ppp.tile([128, 64], f32, tag="pp")
                nc.tensor.matmul(b2_ps[:, 0:CLS], ones_sb[0:1, :],
                                 b2_sb[0:1, :], start=True, stop=True)
                b2bc = cp.tile([128, CLS], f32, tag="b2bc")
                nc.scalar.activation(b2bc[:], b2_ps[:, 0:CLS], ACTF.Copy)

            # =============== Layer 2 ===============
            with (
                tc.tile_pool(name="aggps2", bufs=3, space="PSUM") as aggpp2,
                tc.tile_pool(name="op", bufs=2, space="PSUM") as opp,
            ):
                if phases == 2:
                    for t in range(TPC):
                        res = smp.tile([128, CLS], f32, tag="res")
                        nc.vector.tensor_copy(res[:], h1T0[:, t * 128:t * 128 + CLS])
                        rows = NPC - t * 128 if t == TPC - 1 else 128
                        nc.sync.dma_start(out_h[t * 128:t * 128 + rows, :], res[0:rows, :])
                for g in (range(NG) if phases >= 3 else []):
                    grp = groups[g]
                    msg = msgp.tile([128, max_gch * 128], bf16, tag="msg")
                    msg3 = msg[:].rearrange("p (c e) -> p c e", e=F)
                    gathers(grp, p_full[0:SPLIT, :], p_full[SPLIT:N, :], msg3)
                    for tl, t in enumerate(grp["tiles"]):
                        agg_ps = aggpp2.tile([128, 128], f32, tag="agg2")
                        nonempty = agg_tile_chunks(grp, t, msg3, agg_ps[:])
                        ts = slice(t * 128, (t + 1) * 128)
                        o_ps = opp.tile([128, 64], f32, tag="op")
                        nc.tensor.matmul(o_ps[:, 0:CLS], h1T0[:, ts],
                                         w2r_sb[:, 0:CLS], start=True,
                                         stop=False)
                        nc.tensor.matmul(o_ps[:, 0:CLS], h1T1[:, ts],
                                         w2r_sb[:, CLS:2 * CLS], start=False,
                                         stop=True)
                        s_sb = smp.tile([128, CLS], f32, tag="s")
                        if nonempty:
                            nc.vector.tensor_scalar(
                                s_sb[:],
                                agg_ps[:, 0:CLS],
                                inv_sb[:, t:t + 1], None, ALU.mult)
                        else:
                            nc.vector.memset(s_sb[:], 0.0)
                        lg = smp.tile([128, CLS], f32, tag="lg")
                        nc.vector.tensor_tensor(lg[:], o_ps[:, 0:CLS], s_sb[:],
                                                ALU.add)
                        lg2 = smp.tile([128, CLS], f32, tag="lg2")
                        nc.vector.tensor_tensor(lg2[:], lg[:], b2bc[:], ALU.add)
                        mx = smp.tile([128, 1], f32, tag="mx")
                        nc.vector.tensor_reduce(mx[:], lg2[:],
                                                mybir.AxisListType.X, ALU.max)
                        sh = smp.tile([128, CLS], f32, tag="sh")
                        nc.vector.tensor_scalar(sh[:], lg2[:], mx[:, 0:1], None,
                                                ALU.subtract)
                        ex = smp.tile([128, CLS], f32, tag="ex")
                        nc.scalar.activation(ex[:], sh[:], ACTF.Exp)
                        sm = smp.tile([128, 1], f32, tag="sm")
                        nc.vector.tensor_reduce(sm[:], ex[:],
                                                mybir.AxisListType.X, ALU.add)
                        ls = smp.tile([128, 1], f32, tag="ls")
                        nc.scalar.activation(ls[:], sm[:], ACTF.Ln)
                        res = smp.tile([128, CLS], f32, tag="res")
                        nc.vector.tensor_scalar(res[:], sh[:], ls[:, 0:1], None,
                                                ALU.subtract)
                        rows = NPC - t * 128 if t == TPC - 1 else 128
                        nc.sync.dma_start(out_h[t * 128:t * 128 + rows, :],
                                          res[0:rows, :])

    nc.compile()
    return nc




def _make_in_maps(inputs, gidx_all, dstv_all, degp_all, xown_all):
    x = np.asarray(inputs["x"], np.float32)
    xbf = np.asarray(x, ml_dtypes.bfloat16)
    w2lf = np.asarray(inputs["W2_l"], np.float32)
    w2rf = np.asarray(inputs["W2_r"], np.float32)
    w2l = np.ascontiguousarray(np.concatenate([w2lf[:128, :], w2lf[128:, :]], axis=1))
    w2r = np.ascontiguousarray(np.concatenate([w2rf[:128, :], w2rf[128:, :]], axis=1))
    b1c = np.ascontiguousarray(np.asarray(inputs["b1"], np.float32).reshape(2, 128).T)
    b2r = np.ascontiguousarray(np.asarray(inputs["b2"], np.float32).reshape(1, CLS))
    w1l = np.ascontiguousarray(np.asarray(inputs["W1_l"], np.float32))
    w1r = np.ascontiguousarray(np.asarray(inputs["W1_r"], np.float32))
    in_maps = []
    for c in range(CORES):
        in_maps.append({
            "xbf": xbf,
            "xown": xown_all[c],
            "gidx": gidx_all[c],
            "dstv": dstv_all[c],
            "degp": degp_all[c],
            "w1l": w1l, "w1r": w1r, "w2l": w2l, "w2r": w2r,
            "b1c": b1c, "b2r": b2r,
            "iotab": IOTA_BF, "ident": IDENT_F32,
        })
    return in_maps


def _run(inputs, trace=False):
    x = np.asarray(inputs["x"], np.float32)
    edge_index = np.asarray(inputs["edge_index"])
    sched, gidx_all, dstv_all, degp_all, xown_all = _host_prep(x, edge_index)
    nc = _build(sched)
    in_maps = _make_in_maps(inputs, gidx_all, dstv_all, degp_all, xown_all)
    res = run_bass_kernel_spmd(nc, in_maps, core_ids=list(range(CORES)),
                               trace=trace)
    out = np.concatenate([r["out"] for r in res.results], axis=0)
    return out, res


def kernel(**inputs):
    out, _ = _run(inputs, trace=False)
    return out



# revision 8
# speedup vs baseline: 4.3551x; 4.3551x over previous
"""GraphSAGE 2-layer forward on 8 TRN2 NeuronCores.

Strategy (graph/data parallel per sharding hint):
- Nodes dst-sharded across 8 cores (6250 nodes/core, 49 tiles of 128).
- x is sharded host-side: each core receives only its own 6250-row bf16
  shard; the full bf16 feature table is assembled ON DEVICE via AllGather
  (x is the dominant transfer, so this cuts host->device traffic 8x).
- Host sorts edges by dst, buckets per (core, dst-tile), splits by src<32768
  (dma_gather idx is int16) and pads each bucket to 128-slot chunks.
- L1: gather x_bf16[src] rows (256B) from the AllGathered table via
  gpsimd.dma_gather; scatter-add via one-hot matmuls into PSUM (one-hot
  built on DVE with on-device iota + is_equal against per-slot dst values);
  mean via per-partition inv-degree scale; dense W1_l/W1_r matmuls (bf16)
  fused bias+relu on ACT.
- h kept transposed [hid, nodes] bf16 in SBUF; p = h @ W2_l computed
  row-major, AllGathered (bf16, 128-col padded rows) so every core can
  gather p[src].
- L2: same gather/scatter machinery on p; + h @ W2_r + b2; log_softmax
  along the free dim; bf16 out, upcast on host.
- Index/one-hot side tables are minimized: dma_gather idx is shipped as a
  single [16, W] block and replicated to 128 partitions on device; dst
  slot values ship as int8; iota/identity matrices are generated on device.
"""

import numpy as np
import ml_dtypes

import concourse.bacc as bacc
import concourse.bass as bass
import concourse.mybir as mybir
import concourse.tile as tile
from concourse.bass_utils import run_bass_kernel_spmd

N = 50000
F = 128
HID = 256
CLS = 47
CORES = 8
NPC = N // CORES           # 6250
TPC = (NPC + 127) // 128   # 49 tiles per core
SPLIT = 32768              # int16 index limit for dma_gather
GPT = 7                    # dst-tiles per gather group
NG = (TPC + GPT - 1) // GPT

f32 = mybir.dt.float32
bf16 = mybir.dt.bfloat16
i16 = mybir.dt.int16
i8 = mybir.dt.int8
ALU = mybir.AluOpType
ACTF = mybir.ActivationFunctionType


def _host_prep(x, edge_index):
    src = np.asarray(edge_index[0], np.int64)
    dst = np.asarray(edge_index[1], np.int64)
    deg = np.bincount(dst, minlength=N).astype(np.float32)

    order = np.argsort(dst, kind="stable")
    src_s = src[order]
    dst_s = dst[order]
    bounds = np.searchsorted(dst_s, np.arange(0, N + 1, NPC))

    seg_idx = {}
    cnt = np.zeros((CORES, TPC, 2), np.int64)
    for c in range(CORES):
        sl = slice(bounds[c], bounds[c + 1])
        sc = src_s[sl]
        dcl = dst_s[sl] - c * NPC
        tt = dcl >> 7
        t_ord = np.argsort(tt, kind="stable")
        sc, dcl, tt = sc[t_ord], dcl[t_ord], tt[t_ord]
        tb = np.searchsorted(tt, np.arange(TPC + 1))
        for t in range(TPC):
            s2 = slice(tb[t], tb[t + 1])
            s_t = sc[s2]
            d_t = dcl[s2] & 127
            lo = s_t < SPLIT
            seg_idx[(c, t, 0)] = (s_t[lo], d_t[lo])
            seg_idx[(c, t, 1)] = (s_t[~lo] - SPLIT, d_t[~lo])
            cnt[c, t, 0] = int(lo.sum())
            cnt[c, t, 1] = int((~lo).sum())

    # chunk counts, uniform across cores (SPMD single program)
    nch = np.ceil(cnt / 128.0).astype(np.int64).max(axis=0)  # [TPC, 2]

    groups = []
    chunk_ptr = 0
    for g in range(NG):
        tiles = list(range(g * GPT, min((g + 1) * GPT, TPC)))
        seg_chunks = {0: {}, 1: {}}
        base = chunk_ptr
        for s in (0, 1):
            for t in tiles:
                seg_chunks[s][t] = (chunk_ptr, int(nch[t, s]))
                chunk_ptr += int(nch[t, s])
        groups.append(dict(tiles=tiles, seg_chunks=seg_chunks, base=base,
                           nchunks=chunk_ptr - base))
    tot_ch = chunk_ptr
    W = tot_ch * 8  # idx columns: 128 slots/chunk / 16

    gidx_all, dstv_all, degp_all, xsh_all = [], [], [], []
    xbf = np.asarray(x, ml_dtypes.bfloat16)
    for c in range(CORES):
        gi = np.zeros((16, W), np.int16)
        dv = np.full((128, tot_ch), -1, np.int8)
        for t in range(TPC):
            g = t // GPT
            for s in (0, 1):
                c0, ncks = groups[g]["seg_chunks"][s][t]
                if ncks == 0:
                    continue
                iv, dl = seg_idx[(c, t, s)]
                S = ncks * 128
                ivp = np.zeros(S, np.int64)
                ivp[: len(iv)] = iv
                dvp = np.full(S, -1, np.int64)
                dvp[: len(dl)] = dl
                gi[:, c0 * 8:(c0 + ncks) * 8] = ivp.reshape(-1, 16).T
                dv[:, c0:c0 + ncks] = dvp.reshape(ncks, 128).T
        gidx_all.append(gi)
        dstv_all.append(dv)
        dpc = np.ones(TPC * 128, np.float32)
        dpc[:NPC] = deg[c * NPC:(c + 1) * NPC]
        degp_all.append(np.ascontiguousarray(dpc.reshape(TPC, 128).T))
        xsh_all.append(np.ascontiguousarray(xbf[c * NPC:(c + 1) * NPC]))

    sched = dict(groups=groups, tot_ch=tot_ch, W=W,
                 max_gch=max(g["nchunks"] for g in groups))
    return sched, gidx_all, dstv_all, degp_all, xsh_all


def _build(sched):
    groups, tot_ch, W = sched["groups"], sched["tot_ch"], sched["W"]
    max_gch = sched["max_gch"]

    nc = bacc.Bacc("TRN2", num_devices=CORES)
    xsh_h = nc.declare_dram_parameter("xsh", [NPC, F], bf16, False)
    gidx_h = nc.declare_dram_parameter("gidx", [16, W], i16, False)
    dstv_h = nc.declare_dram_parameter("dstv", [128, tot_ch], i8, False)
    degp_h = nc.declare_dram_parameter("degp", [128, TPC], f32, False)
    w1l_h = nc.declare_dram_parameter("w1l", [F, HID], bf16, False)
    w1r_h = nc.declare_dram_parameter("w1r", [F, HID], bf16, False)
    w2l_h = nc.declare_dram_parameter("w2l", [128, 2 * CLS], bf16, False)
    w2r_h = nc.declare_dram_parameter("w2r", [128, 2 * CLS], bf16, False)
    b1_h = nc.declare_dram_parameter("b1c", [128, 2], f32, False)
    b2_h = nc.declare_dram_parameter("b2r", [1, CLS], f32, False)
    out_h = nc.declare_dram_parameter("out", [NPC, CLS], bf16, True)

    x_loc = nc.dram_tensor("x_loc", [NPC, F], bf16)
    x_full = nc.dram_tensor("x_full", [N, F], bf16, addr_space="Shared")
    p_loc = nc.dram_tensor("p_loc", [NPC, 128], bf16)
    p_full = nc.dram_tensor("p_full", [N, 128], bf16, addr_space="Shared")

    with tile.TileContext(nc) as tc:
        with (
            tc.tile_pool(name="const", bufs=1) as cp,
            tc.tile_pool(name="msg", bufs=2) as msgp,
            tc.tile_pool(name="oh", bufs=6) as ohp,
            tc.tile_pool(name="sb", bufs=3) as sbp,
            tc.tile_pool(name="small", bufs=4) as smp,
        ):
            # ---- assemble full bf16 feature table on device ----
            # (collectives cannot read IO tensors: stage via internal dram)
            nc.sync.dma_start(x_loc.ap(), xsh_h[:, :])
            nc.gpsimd.collective_compute(
                "AllGather", ALU.bypass,
                replica_groups=[list(range(CORES))],
                ins=[x_loc.ap().opt()], outs=[x_full.ap().opt()])

            # ---- persistent tiles ----
            idx_sb = cp.tile([128, W], i16, tag="idx")
            nc.sync.dma_start(idx_sb[0:16, :], gidx_h[:, :])
            # dma_gather wants the idx block replicated across the 8 Q7
            # cores (16 partitions each) — replicate on device.
            nc.sync.dma_start(idx_sb[16:32, :], idx_sb[0:16, :])
            nc.sync.dma_start(idx_sb[32:64, :], idx_sb[0:32, :])
            nc.sync.dma_start(idx_sb[64:128, :], idx_sb[0:64, :])

            dstv8_sb = cp.tile([128, tot_ch], i8, tag="dstv8")
            nc.sync.dma_start(dstv8_sb[:], dstv_h[:, :])
            dstv_sb = cp.tile([128, tot_ch], f32, tag="dstv")
            nc.vector.tensor_copy(dstv_sb[:], dstv8_sb[:])

            w1l_sb = cp.tile([F, HID], bf16, tag="w1l")
            nc.sync.dma_start(w1l_sb[:], w1l_h[:, :])
            w1r_sb = cp.tile([F, HID], bf16, tag="w1r")
            nc.sync.dma_start(w1r_sb[:], w1r_h[:, :])
            w2l_sb = cp.tile([128, 2 * CLS], bf16, tag="w2l")
            nc.sync.dma_start(w2l_sb[:], w2l_h[:, :])
            w2r_sb = cp.tile([128, 2 * CLS], bf16, tag="w2r")
            nc.sync.dma_start(w2r_sb[:], w2r_h[:, :])
            b1_sb = cp.tile([128, 2], f32, tag="b1")
            nc.sync.dma_start(b1_sb[:], b1_h[:, :])
            b2_sb = cp.tile([1, CLS], f32, tag="b2")
            nc.sync.dma_start(b2_sb[:], b2_h[:, :])
            deg_sb = cp.tile([128, TPC], f32, tag="deg")
            nc.sync.dma_start(deg_sb[:], degp_h[:, :])

            inv_sb = cp.tile([128, TPC], f32, tag="inv")
            nc.vector.tensor_scalar(inv_sb[:], deg_sb[:], 1.0, None, ALU.max)
            nc.vector.reciprocal(inv_sb[:], inv_sb[:])

            # iota row (0..127 along free dim) and identity, built on device
            iota_bf = cp.tile([128, 128], bf16, tag="iotabf")
            nc.gpsimd.iota(iota_bf[:], [[1, 128]], base=0,
                           channel_multiplier=0,
                           allow_small_or_imprecise_dtypes=True)
            pcol_f = cp.tile([128, 1], f32, tag="pcol")
            nc.gpsimd.iota(pcol_f[:], [[1, 1]], base=0,
                           channel_multiplier=1,
                           allow_small_or_imprecise_dtypes=True)
            ident_bf = cp.tile([128, 128], bf16, tag="ident")
            nc.vector.tensor_scalar(ident_bf[:], iota_bf[:],
                                    pcol_f[:, 0:1], None, ALU.is_equal)
            ones_sb = cp.tile([1, 128], f32, tag="ones")
            nc.vector.memset(ones_sb[:], 1.0)

            h1T0 = cp.tile([128, TPC * 128], bf16, tag="h1a")
            h1T1 = cp.tile([128, TPC * 128], bf16, tag="h1b")

            def gathers(group, table_lo, table_hi, msg3):
                """Issue lo/hi dma_gather for one group into msg3 [128,C,128]."""
                base = group["base"]
                n_lo = sum(n for (_, n) in group["seg_chunks"][0].values())
                n_hi = sum(n for (_, n) in group["seg_chunks"][1].values())
                if n_lo:
                    S = n_lo * 128
                    nc.gpsimd.dma_gather(
                        msg3[:, 0:n_lo, :], table_lo,
                        idx_sb[:, base * 8:(base + n_lo) * 8],
                        S, S, F, single_packet=False)
                if n_hi:
                    S = n_hi * 128
                    nc.gpsimd.dma_gather(
                        msg3[:, n_lo:n_lo + n_hi, :], table_hi,
                        idx_sb[:, (base + n_lo) * 8:(base + n_lo + n_hi) * 8],
                        S, S, F, single_packet=False)

            def agg_tile_chunks(group, t, msg3, psl):
                """One-hot matmuls accumulating agg for dst-tile t."""
                base = group["base"]
                lo0, nlo = group["seg_chunks"][0][t]
                hi0, nhi = group["seg_chunks"][1][t]
                gcs = [lo0 + k for k in range(nlo)] + \
                      [hi0 + k for k in range(nhi)]
                for i, gc in enumerate(gcs):
                    oh = ohp.tile([128, 128], bf16, tag="oh")
                    nc.vector.tensor_scalar(oh[:], iota_bf[:],
                                            dstv_sb[:, gc:gc + 1], None,
                                            ALU.is_equal)
                    nc.tensor.matmul(psl, oh[:], msg3[:, gc - base, :],
                                     start=(i == 0), stop=(i == len(gcs) - 1))
                return len(gcs) > 0

            # =============== Layer 1 ===============
            with (
                tc.tile_pool(name="aggps", bufs=3, space="PSUM") as aggpp,
                tc.tile_pool(name="tp", bufs=2, space="PSUM") as tpp,
                tc.tile_pool(name="zp", bufs=2, space="PSUM") as zpp,
            ):
                for g in range(NG):
                    grp = groups[g]
                    msg = msgp.tile([128, max_gch * 128], bf16, tag="msg")
                    msg3 = msg[:].rearrange("p (c e) -> p c e", e=F)
                    gathers(grp, x_full[0:SPLIT, :], x_full[SPLIT:N, :], msg3)
                    for tl, t in enumerate(grp["tiles"]):
                        agg_ps = aggpp.tile([128, 128], f32, tag="agg")
                        nonempty = agg_tile_chunks(grp, t, msg3, agg_ps[:])
                        mean = sbp.tile([128, 128], bf16, tag="mean")
                        if nonempty:
                            nc.vector.tensor_scalar(
                                mean[:], agg_ps[:],
                                inv_sb[:, t:t + 1], None, ALU.mult)
                        else:
                            nc.vector.memset(mean[:], 0.0)
                        mt_ps = tpp.tile([128, 128], bf16, tag="tp")
                        nc.tensor.transpose(mt_ps[:], mean[:], ident_bf[:])
                        meanT = sbp.tile([128, 128], bf16, tag="meanT")
                        nc.scalar.activation(meanT[:], mt_ps[:], ACTF.Copy)
                        xo = sbp.tile([128, 128], bf16, tag="xo")
                        rows = NPC - t * 128 if t == TPC - 1 else 128
                        if rows < 128:
                            nc.vector.memset(xo[:], 0.0)
                        nc.sync.dma_start(xo[0:rows, :],
                                          xsh_h[t * 128:t * 128 + rows, :])
                        xt_ps = tpp.tile([128, 128], bf16, tag="tp")
                        nc.tensor.transpose(xt_ps[:], xo[:], ident_bf[:])
                        xoT = sbp.tile([128, 128], bf16, tag="xoT")
                        nc.scalar.activation(xoT[:], xt_ps[:], ACTF.Copy)
                        z_ps = zpp.tile([128, 256], f32, tag="z")
                        for h, h1T in ((0, h1T0), (1, h1T1)):
                            zs = z_ps[:, h * 128:(h + 1) * 128]
                            nc.tensor.matmul(zs, w1l_sb[:, h * 128:(h + 1) * 128],
                                             meanT[:], start=True, stop=False)
                            nc.tensor.matmul(zs, w1r_sb[:, h * 128:(h + 1) * 128],
                                             xoT[:], start=False, stop=True)
                            nc.scalar.activation(h1T[:, t * 128:(t + 1) * 128],
                                                 zs, ACTF.Relu,
                                                 bias=b1_sb[:, h:h + 1],
                                                 scale=1.0)

            # =============== p = h @ W2_l, AllGather ===============
            with tc.tile_pool(name="pp", bufs=2, space="PSUM") as ppp:
                for t in range(TPC):
                    ts = slice(t * 128, (t + 1) * 128)
                    pp_ps = ppp.tile([128, 64], f32, tag="pp")
                    nc.tensor.matmul(pp_ps[:, 0:CLS], h1T0[:, ts],
                                     w2l_sb[:, 0:CLS], start=True, stop=False)
                    nc.tensor.matmul(pp_ps[:, 0:CLS], h1T1[:, ts],
                                     w2l_sb[:, CLS:2 * CLS], start=False,
                                     stop=True)
                    psb = sbp.tile([128, 128], bf16, tag="psb")
                    nc.vector.memset(psb[:, CLS:128], 0.0)
                    nc.scalar.activation(psb[:, 0:CLS], pp_ps[:, 0:CLS],
                                         ACTF.Copy)
                    rows = NPC - t * 128 if t == TPC - 1 else 128
                    nc.sync.dma_start(p_loc[t * 128:t * 128 + rows, :],
                                      psb[0:rows, :])

                nc.gpsimd.collective_compute(
                    "AllGather", ALU.bypass,
                    replica_groups=[list(range(CORES))],
                    ins=[p_loc.ap().opt()], outs=[p_full.ap().opt()])

                # b2 broadcast across partitions via rank-1 matmul
                b2_ps = ppp.tile([128, 64], f32, tag="pp")
                nc.tensor.matmul(b2_ps[:, 0:CLS], ones_sb[0:1, :],
                                 b2_sb[0:1, :], start=True, stop=True)
                b2bc = cp.tile([128, CLS], f32, tag="b2bc")
                nc.scalar.activation(b2bc[:], b2_ps[:, 0:CLS], ACTF.Copy)

            # =============== Layer 2 ===============
            with (
                tc.tile_pool(name="aggps2", bufs=3, space="PSUM") as aggpp2,
                tc.tile_pool(name="op", bufs=2, space="PSUM") as opp,
            ):
                for g in range(NG):
                    grp = groups[g]
                    msg = msgp.tile([128, max_gch * 128], bf16, tag="msg")
                    msg3 = msg[:].rearrange("p (c e) -> p c e", e=F)
                    gathers(grp, p_full[0:SPLIT, :], p_full[SPLIT:N, :], msg3)
                    for tl, t in enumerate(grp["tiles"]):
                        agg_ps = aggpp2.tile([128, 128], f32, tag="agg2")
                        nonempty = agg_tile_chunks(grp, t, msg3, agg_ps[:])
                        ts = slice(t * 128, (t + 1) * 128)
                        o_ps = opp.tile([128, 64], f32, tag="op")
                        nc.tensor.matmul(o_ps[:, 0:CLS], h1T0[:, ts],
                                         w2r_sb[:, 0:CLS], start=True,
                                         stop=False)
                        nc.tensor.matmul(o_ps[:, 0:CLS], h1T1[:, ts],
                                         w2r_sb[:, CLS:2 * CLS], start=False,
                                         stop=True)
                        s_sb = smp.tile([128, CLS], f32, tag="s")
                        if nonempty:
                            nc.vector.tensor_scalar(
                                s_sb[:],
                                agg_ps[:, 0:CLS],
                                inv_sb[:, t:t + 1], None, ALU.mult)
                        else:
                            nc.vector.memset(s_sb[:], 0.0)
                        lg = smp.tile([128, CLS], f32, tag="lg")
                        nc.vector.tensor_tensor(lg[:], o_ps[:, 0:CLS], s_sb[:],
                                                ALU.add)
                        lg2 = smp.tile([128, CLS], f32, tag="lg2")
                        nc.vector.tensor_tensor(lg2[:], lg[:], b2bc[:], ALU.add)
                        mx = smp.tile([128, 1], f32, tag="mx")
                        nc.vector.tensor_reduce(mx[:], lg2[:],
                                                mybir.AxisListType.X, ALU.max)
                        sh = smp.tile([128, CLS], f32, tag="sh")
                        nc.vector.tensor_scalar(sh[:], lg2[:], mx[:, 0:1], None,
                                                ALU.subtract)
                        ex = smp.tile([128, CLS], f32, tag="ex")
                        nc.scalar.activation(ex[:], sh[:], ACTF.Exp)
                        sm = smp.tile([128, 1], f32, tag="sm")
                        nc.vector.tensor_reduce(sm[:], ex[:],
                                                mybir.AxisListType.X, ALU.add)
                        ls = smp.tile([128, 1], f32, tag="ls")
                        nc.scalar.activation(ls[:], sm[:], ACTF.Ln)
                        res = smp.tile([128, CLS], bf16, tag="res")
                        nc.vector.tensor_scalar(res[:], sh[:], ls[:, 0:1], None,
                                                ALU.subtract)
                        rows = NPC - t * 128 if t == TPC - 1 else 128
                        nc.sync.dma_start(out_h[t * 128:t * 128 + rows, :],
                                          res[0:rows, :])

    nc.compile()
    return nc


def _make_in_maps(inputs, gidx_all, dstv_all, degp_all, xsh_all):
    w1l = np.asarray(np.asarray(inputs["W1_l"], np.float32),
                     ml_dtypes.bfloat16)
    w1r = np.asarray(np.asarray(inputs["W1_r"], np.float32),
                     ml_dtypes.bfloat16)
    w2lf = np.asarray(inputs["W2_l"], np.float32)
    w2rf = np.asarray(inputs["W2_r"], np.float32)
    w2l = np.ascontiguousarray(
        np.concatenate([w2lf[:128, :], w2lf[128:, :]], axis=1)).astype(
            ml_dtypes.bfloat16)
    w2r = np.ascontiguousarray(
        np.concatenate([w2rf[:128, :], w2rf[128:, :]], axis=1)).astype(
            ml_dtypes.bfloat16)
    b1c = np.ascontiguousarray(
        np.asarray(inputs["b1"], np.float32).reshape(2, 128).T)
    b2r = np.ascontiguousarray(
        np.asarray(inputs["b2"], np.float32).reshape(1, CLS))
    in_maps = []
    for c in range(CORES):
        in_maps.append({
            "xsh": xsh_all[c],
            "gidx": gidx_all[c],
            "dstv": dstv_all[c],
            "degp": degp_all[c],
            "w1l": w1l, "w1r": w1r, "w2l": w2l, "w2r": w2r,
            "b1c": b1c, "b2r": b2r,
        })
    return in_maps


def _run(inputs, trace=False):
    x = np.asarray(inputs["x"], np.float32)
    edge_index = np.asarray(inputs["edge_index"])
    sched, gidx_all, dstv_all, degp_all, xsh_all = _host_prep(x, edge_index)
    nc = _build(sched)
    in_maps = _make_in_maps(inputs, gidx_all, dstv_all, degp_all, xsh_all)
    res = run_bass_kernel_spmd(nc, in_maps, core_ids=list(range(CORES)),
                               trace=trace)
    out = np.concatenate(
        [np.asarray(r["out"], np.float32) for r in res.results], axis=0)
    return out, res


def kernel(**inputs):
    out, _ = _run(inputs, trace=False)
    return out


# revision 9
# speedup vs baseline: 5.1665x; 1.1863x over previous
"""GraphSAGE 2-layer forward on 8 TRN2 NeuronCores.

Strategy (graph/data parallel per sharding hint):
- Nodes dst-sharded across 8 cores (6250 nodes/core, 49 tiles of 128).
- x is sharded host-side: each core receives only its own 6250-row bf16
  shard; the full bf16 feature table is assembled ON DEVICE via AllGather
  (x is the dominant transfer, so this cuts host->device traffic 8x).
- Host sorts edges by dst, buckets per (core, dst-tile), splits by src<32768
  (dma_gather idx is int16) and pads each bucket to 128-slot chunks.
- L1: gather x_bf16[src] rows (256B) from the AllGathered table via
  gpsimd.dma_gather; scatter-add via one-hot matmuls into PSUM (one-hot
  built on DVE with on-device iota + is_equal against per-slot dst values);
  mean via per-partition inv-degree scale; dense W1_l/W1_r matmuls (bf16)
  fused bias+relu on ACT.
- h kept transposed [hid, nodes] bf16 in SBUF; p = h @ W2_l computed
  row-major, AllGathered (bf16, 128-col padded rows) so every core can
  gather p[src].
- L2: same gather/scatter machinery on p; + h @ W2_r + b2; log_softmax
  along the free dim; bf16 out, upcast on host.
- Index/one-hot side tables are minimized: dma_gather idx is shipped as a
  single [16, W] block and replicated to 128 partitions on device; dst
  slot values ship as int8; iota/identity matrices are generated on device.
"""

import numpy as np
import ml_dtypes

import concourse.bacc as bacc
import concourse.bass as bass
import concourse.mybir as mybir
import concourse.tile as tile
from concourse.bass_utils import run_bass_kernel_spmd

N = 50000
F = 128
HID = 256
CLS = 47
CORES = 8
NPC = N // CORES           # 6250
TPC = (NPC + 127) // 128   # 49 tiles per core
SPLIT = 32768              # int16 index limit for dma_gather
GPT = 7                    # dst-tiles per gather group
NG = (TPC + GPT - 1) // GPT

f32 = mybir.dt.float32
bf16 = mybir.dt.bfloat16
i16 = mybir.dt.int16
i8 = mybir.dt.int8
ALU = mybir.AluOpType
ACTF = mybir.ActivationFunctionType


def _host_prep(x, edge_index):
    src = np.asarray(edge_index[0], np.int64)
    dst = np.asarray(edge_index[1], np.int64)
    deg = np.bincount(dst, minlength=N).astype(np.float32)

    order = np.argsort(dst, kind="stable")
    src_s = src[order]
    dst_s = dst[order]
    bounds = np.searchsorted(dst_s, np.arange(0, N + 1, NPC))

    seg_idx = {}
    cnt = np.zeros((CORES, TPC, 2), np.int64)
    for c in range(CORES):
        sl = slice(bounds[c], bounds[c + 1])
        sc = src_s[sl]
        dcl = dst_s[sl] - c * NPC
        tt = dcl >> 7
        t_ord = np.argsort(tt, kind="stable")
        sc, dcl, tt = sc[t_ord], dcl[t_ord], tt[t_ord]
        tb = np.searchsorted(tt, np.arange(TPC + 1))
        for t in range(TPC):
            s2 = slice(tb[t], tb[t + 1])
            s_t = sc[s2]
            d_t = dcl[s2] & 127
            lo = s_t < SPLIT
            seg_idx[(c, t, 0)] = (s_t[lo], d_t[lo])
            seg_idx[(c, t, 1)] = (s_t[~lo] - SPLIT, d_t[~lo])
            cnt[c, t, 0] = int(lo.sum())
            cnt[c, t, 1] = int((~lo).sum())

    # chunk counts, uniform across cores (SPMD single program)
    nch = np.ceil(cnt / 128.0).astype(np.int64).max(axis=0)  # [TPC, 2]

    groups = []
    chunk_ptr = 0
    for g in range(NG):
        tiles = list(range(g * GPT, min((g + 1) * GPT, TPC)))
        seg_chunks = {0: {}, 1: {}}
        base = chunk_ptr
        for s in (0, 1):
            for t in tiles:
                seg_chunks[s][t] = (chunk_ptr, int(nch[t, s]))
                chunk_ptr += int(nch[t, s])
        groups.append(dict(tiles=tiles, seg_chunks=seg_chunks, base=base,
                           nchunks=chunk_ptr - base))
    tot_ch = chunk_ptr
    W = tot_ch * 8  # idx columns: 128 slots/chunk / 16

    gidx_all, dstv_all, degp_all, xsh_all = [], [], [], []
    xbf = np.asarray(x, ml_dtypes.bfloat16)
    for c in range(CORES):
        gi = np.zeros((16, W), np.int16)
        dv = np.full((128, tot_ch), -1, np.int8)
        for t in range(TPC):
            g = t // GPT
            for s in (0, 1):
                c0, ncks = groups[g]["seg_chunks"][s][t]
                if ncks == 0:
                    continue
                iv, dl = seg_idx[(c, t, s)]
                S = ncks * 128
                ivp = np.zeros(S, np.int64)
                ivp[: len(iv)] = iv
                dvp = np.full(S, -1, np.int64)
                dvp[: len(dl)] = dl
                gi[:, c0 * 8:(c0 + ncks) * 8] = ivp.reshape(-1, 16).T
                dv[:, c0:c0 + ncks] = dvp.reshape(ncks, 128).T
        gidx_all.append(gi)
        dstv_all.append(dv)
        dpc = np.ones(TPC * 128, np.float32)
        dpc[:NPC] = deg[c * NPC:(c + 1) * NPC]
        degp_all.append(np.ascontiguousarray(dpc.reshape(TPC, 128).T))
        xsh_all.append(np.ascontiguousarray(xbf[c * NPC:(c + 1) * NPC]))

    sched = dict(groups=groups, tot_ch=tot_ch, W=W,
                 max_gch=max(g["nchunks"] for g in groups))
    return sched, gidx_all, dstv_all, degp_all, xsh_all


def _build(sched):
    groups, tot_ch, W = sched["groups"], sched["tot_ch"], sched["W"]
    max_gch = sched["max_gch"]

    nc = bacc.Bacc("TRN2", num_devices=CORES)
    xsh_h = nc.declare_dram_parameter("xsh", [NPC, F], bf16, False)
    gidx_h = nc.declare_dram_parameter("gidx", [16, W], i16, False)
    dstv_h = nc.declare_dram_parameter("dstv", [128, tot_ch], i8, False)
    degp_h = nc.declare_dram_parameter("degp", [128, TPC], f32, False)
    w1l_h = nc.declare_dram_parameter("w1l", [F, HID], bf16, False)
    w1r_h = nc.declare_dram_parameter("w1r", [F, HID], bf16, False)
    w2l_h = nc.declare_dram_parameter("w2l", [128, 2 * CLS], bf16, False)
    w2r_h = nc.declare_dram_parameter("w2r", [128, 2 * CLS], bf16, False)
    b1_h = nc.declare_dram_parameter("b1c", [128, 2], f32, False)
    b2_h = nc.declare_dram_parameter("b2r", [1, CLS], f32, False)
    out_h = nc.declare_dram_parameter("out", [NPC, CLS], bf16, True)

    x_loc = nc.dram_tensor("x_loc", [NPC, F], bf16)
    x_full = nc.dram_tensor("x_full", [N, F], bf16, addr_space="Shared")
    p_loc = nc.dram_tensor("p_loc", [NPC, 128], bf16)
    p_full = nc.dram_tensor("p_full", [N, 128], bf16, addr_space="Shared")

    with tile.TileContext(nc) as tc:
        with (
            tc.tile_pool(name="const", bufs=1) as cp,
            tc.tile_pool(name="msg", bufs=2) as msgp,
            tc.tile_pool(name="oh", bufs=6) as ohp,
            tc.tile_pool(name="sb", bufs=3) as sbp,
            tc.tile_pool(name="small", bufs=4) as smp,
        ):
            # ---- assemble full bf16 feature table on device ----
            # (collectives cannot read IO tensors: stage via internal dram)
            nc.sync.dma_start(x_loc.ap(), xsh_h[:, :])
            nc.gpsimd.collective_compute(
                "AllGather", ALU.bypass,
                replica_groups=[list(range(CORES))],
                ins=[x_loc.ap().opt()], outs=[x_full.ap().opt()])

            # ---- persistent tiles ----
            idx_sb = cp.tile([128, W], i16, tag="idx")
            nc.sync.dma_start(idx_sb[0:16, :], gidx_h[:, :])
            # dma_gather wants the idx block replicated across the 8 Q7
            # cores (16 partitions each) — replicate on device.
            nc.sync.dma_start(idx_sb[16:32, :], idx_sb[0:16, :])
            nc.sync.dma_start(idx_sb[32:64, :], idx_sb[0:32, :])
            nc.sync.dma_start(idx_sb[64:128, :], idx_sb[0:64, :])

            dstv8_sb = cp.tile([128, tot_ch], i8, tag="dstv8")
            nc.sync.dma_start(dstv8_sb[:], dstv_h[:, :])
            dstv_sb = cp.tile([128, tot_ch], f32, tag="dstv")
            nc.vector.tensor_copy(dstv_sb[:], dstv8_sb[:])

            w1l_sb = cp.tile([F, HID], bf16, tag="w1l")
            nc.sync.dma_start(w1l_sb[:], w1l_h[:, :])
            w1r_sb = cp.tile([F, HID], bf16, tag="w1r")
            nc.sync.dma_start(w1r_sb[:], w1r_h[:, :])
            w2l_sb = cp.tile([128, 2 * CLS], bf16, tag="w2l")
            nc.sync.dma_start(w2l_sb[:], w2l_h[:, :])
            w2r_sb = cp.tile([128, 2 * CLS], bf16, tag="w2r")
            nc.sync.dma_start(w2r_sb[:], w2r_h[:, :])
            b1_sb = cp.tile([128, 2], f32, tag="b1")
            nc.sync.dma_start(b1_sb[:], b1_h[:, :])
            b2_sb = cp.tile([1, CLS], f32, tag="b2")
            nc.sync.dma_start(b2_sb[:], b2_h[:, :])
            deg_sb = cp.tile([128, TPC], f32, tag="deg")
            nc.sync.dma_start(deg_sb[:], degp_h[:, :])

            invc_sb = cp.tile([128, TPC], f32, tag="invc")
            nc.vector.tensor_scalar(invc_sb[:], deg_sb[:], 1.0, None, ALU.max)
            inv_sb = cp.tile([128, TPC], f32, tag="inv")
            # custom-DVE approx reciprocal: ~18 correct bits, way beyond what
            # the mean-aggregation needs; also keeps the per-NEFF DVE table
            # on the process-cached path.
            nc.vector.reciprocal_approx_fast(inv_sb[:], invc_sb[:])

            # iota row (0..127 along free dim) and identity, built on device
            iota_bf = cp.tile([128, 128], bf16, tag="iotabf")
            nc.gpsimd.iota(iota_bf[:], [[1, 128]], base=0,
                           channel_multiplier=0,
                           allow_small_or_imprecise_dtypes=True)
            pcol_f = cp.tile([128, 1], f32, tag="pcol")
            nc.gpsimd.iota(pcol_f[:], [[1, 1]], base=0,
                           channel_multiplier=1,
                           allow_small_or_imprecise_dtypes=True)
            ident_bf = cp.tile([128, 128], bf16, tag="ident")
            nc.vector.tensor_scalar(ident_bf[:], iota_bf[:],
                                    pcol_f[:, 0:1], None, ALU.is_equal)
            ones_sb = cp.tile([1, 128], f32, tag="ones")
            nc.vector.memset(ones_sb[:], 1.0)

            h1T0 = cp.tile([128, TPC * 128], bf16, tag="h1a")
            h1T1 = cp.tile([128, TPC * 128], bf16, tag="h1b")

            def gathers(group, table_lo, table_hi, msg3):
                """Issue lo/hi dma_gather for one group into msg3 [128,C,128]."""
                base = group["base"]
                n_lo = sum(n for (_, n) in group["seg_chunks"][0].values())
                n_hi = sum(n for (_, n) in group["seg_chunks"][1].values())
                if n_lo:
                    S = n_lo * 128
                    nc.gpsimd.dma_gather(
                        msg3[:, 0:n_lo, :], table_lo,
                        idx_sb[:, base * 8:(base + n_lo) * 8],
                        S, S, F, single_packet=False)
                if n_hi:
                    S = n_hi * 128
                    nc.gpsimd.dma_gather(
                        msg3[:, n_lo:n_lo + n_hi, :], table_hi,
                        idx_sb[:, (base + n_lo) * 8:(base + n_lo + n_hi) * 8],
                        S, S, F, single_packet=False)

            def agg_tile_chunks(group, t, msg3, psl):
                """One-hot matmuls accumulating agg for dst-tile t."""
                base = group["base"]
                lo0, nlo = group["seg_chunks"][0][t]
                hi0, nhi = group["seg_chunks"][1][t]
                gcs = [lo0 + k for k in range(nlo)] + \
                      [hi0 + k for k in range(nhi)]
                for i, gc in enumerate(gcs):
                    oh = ohp.tile([128, 128], bf16, tag="oh")
                    nc.vector.tensor_scalar(oh[:], iota_bf[:],
                                            dstv_sb[:, gc:gc + 1], None,
                                            ALU.is_equal)
                    nc.tensor.matmul(psl, oh[:], msg3[:, gc - base, :],
                                     start=(i == 0), stop=(i == len(gcs) - 1))
                return len(gcs) > 0

            # =============== Layer 1 ===============
            with (
                tc.tile_pool(name="aggps", bufs=3, space="PSUM") as aggpp,
                tc.tile_pool(name="tp", bufs=2, space="PSUM") as tpp,
                tc.tile_pool(name="zp", bufs=2, space="PSUM") as zpp,
            ):
                for g in range(NG):
                    grp = groups[g]
                    msg = msgp.tile([128, max_gch * 128], bf16, tag="msg")
                    msg3 = msg[:].rearrange("p (c e) -> p c e", e=F)
                    gathers(grp, x_full[0:SPLIT, :], x_full[SPLIT:N, :], msg3)
                    for tl, t in enumerate(grp["tiles"]):
                        agg_ps = aggpp.tile([128, 128], f32, tag="agg")
                        nonempty = agg_tile_chunks(grp, t, msg3, agg_ps[:])
                        mean = sbp.tile([128, 128], bf16, tag="mean")
                        if nonempty:
                            nc.vector.tensor_scalar(
                                mean[:], agg_ps[:],
                                inv_sb[:, t:t + 1], None, ALU.mult)
                        else:
                            nc.vector.memset(mean[:], 0.0)
                        mt_ps = tpp.tile([128, 128], bf16, tag="tp")
                        nc.tensor.transpose(mt_ps[:], mean[:], ident_bf[:])
                        meanT = sbp.tile([128, 128], bf16, tag="meanT")
                        nc.scalar.activation(meanT[:], mt_ps[:], ACTF.Copy)
                        xo = sbp.tile([128, 128], bf16, tag="xo")
                        rows = NPC - t * 128 if t == TPC - 1 else 128
                        if rows < 128:
                            nc.vector.memset(xo[:], 0.0)
                        nc.sync.dma_start(xo[0:rows, :],
                                          xsh_h[t * 128:t * 128 + rows, :])
                        xt_ps = tpp.tile([128, 128], bf16, tag="tp")
                        nc.tensor.transpose(xt_ps[:], xo[:], ident_bf[:])
                        xoT = sbp.tile([128, 128], bf16, tag="xoT")
                        nc.scalar.activation(xoT[:], xt_ps[:], ACTF.Copy)
                        z_ps = zpp.tile([128, 256], f32, tag="z")
                        for h, h1T in ((0, h1T0), (1, h1T1)):
                            zs = z_ps[:, h * 128:(h + 1) * 128]
                            nc.tensor.matmul(zs, w1l_sb[:, h * 128:(h + 1) * 128],
                                             meanT[:], start=True, stop=False)
                            nc.tensor.matmul(zs, w1r_sb[:, h * 128:(h + 1) * 128],
                                             xoT[:], start=False, stop=True)
                            nc.scalar.activation(h1T[:, t * 128:(t + 1) * 128],
                                                 zs, ACTF.Relu,
                                                 bias=b1_sb[:, h:h + 1],
                                                 scale=1.0)

            # =============== p = h @ W2_l, AllGather ===============
            with tc.tile_pool(name="pp", bufs=2, space="PSUM") as ppp:
                for t in range(TPC):
                    ts = slice(t * 128, (t + 1) * 128)
                    pp_ps = ppp.tile([128, 64], f32, tag="pp")
                    nc.tensor.matmul(pp_ps[:, 0:CLS], h1T0[:, ts],
                                     w2l_sb[:, 0:CLS], start=True, stop=False)
                    nc.tensor.matmul(pp_ps[:, 0:CLS], h1T1[:, ts],
                                     w2l_sb[:, CLS:2 * CLS], start=False,
                                     stop=True)
                    psb = sbp.tile([128, 128], bf16, tag="psb")
                    nc.vector.memset(psb[:, CLS:128], 0.0)
                    nc.scalar.activation(psb[:, 0:CLS], pp_ps[:, 0:CLS],
                                         ACTF.Copy)
                    rows = NPC - t * 128 if t == TPC - 1 else 128
                    nc.sync.dma_start(p_loc[t * 128:t * 128 + rows, :],
                                      psb[0:rows, :])

                nc.gpsimd.collective_compute(
                    "AllGather", ALU.bypass,
                    replica_groups=[list(range(CORES))],
                    ins=[p_loc.ap().opt()], outs=[p_full.ap().opt()])

                # b2 broadcast across partitions via rank-1 matmul
                b2_ps = ppp.tile([128, 64], f32, tag="pp")
                nc.tensor.matmul(b2_ps[:, 0:CLS], ones_sb[0:1, :],
                                 b2_sb[0:1, :], start=True, stop=True)
                b2bc = cp.tile([128, CLS], f32, tag="b2bc")
                nc.scalar.activation(b2bc[:], b2_ps[:, 0:CLS], ACTF.Copy)

            # =============== Layer 2 ===============
            with (
                tc.tile_pool(name="aggps2", bufs=3, space="PSUM") as aggpp2,
                tc.tile_pool(name="op", bufs=2, space="PSUM") as opp,
            ):
                for g in range(NG):
                    grp = groups[g]
                    msg = msgp.tile([128, max_gch * 128], bf16, tag="msg")
                    msg3 = msg[:].rearrange("p (c e) -> p c e", e=F)
                    gathers(grp, p_full[0:SPLIT, :], p_full[SPLIT:N, :], msg3)
                    for tl, t in enumerate(grp["tiles"]):
                        agg_ps = aggpp2.tile([128, 128], f32, tag="agg2")
                        nonempty = agg_tile_chunks(grp, t, msg3, agg_ps[:])
                        ts = slice(t * 128, (t + 1) * 128)
                        o_ps = opp.tile([128, 64], f32, tag="op")
                        nc.tensor.matmul(o_ps[:, 0:CLS], h1T0[:, ts],
                                         w2r_sb[:, 0:CLS], start=True,
                                         stop=False)
                        nc.tensor.matmul(o_ps[:, 0:CLS], h1T1[:, ts],
                                         w2r_sb[:, CLS:2 * CLS], start=False,
                                         stop=True)
                        s_sb = smp.tile([128, CLS], f32, tag="s")
                        if nonempty:
                            nc.vector.tensor_scalar(
                                s_sb[:],
                                agg_ps[:, 0:CLS],
                                inv_sb[:, t:t + 1], None, ALU.mult)
                        else:
                            nc.vector.memset(s_sb[:], 0.0)
                        lg = smp.tile([128, CLS], f32, tag="lg")
                        nc.vector.tensor_tensor(lg[:], o_ps[:, 0:CLS], s_sb[:],
                                                ALU.add)
                        lg2 = smp.tile([128, CLS], f32, tag="lg2")
                        nc.vector.tensor_tensor(lg2[:], lg[:], b2bc[:], ALU.add)
                        mx = smp.tile([128, 1], f32, tag="mx")
                        nc.vector.tensor_reduce(mx[:], lg2[:],
                                                mybir.AxisListType.X, ALU.max)
                        sh = smp.tile([128, CLS], f32, tag="sh")
                        nc.vector.tensor_scalar(sh[:], lg2[:], mx[:, 0:1], None,
                                                ALU.subtract)
                        ex = smp.tile([128, CLS], f32, tag="ex")
                        nc.scalar.activation(ex[:], sh[:], ACTF.Exp)
                        sm = smp.tile([128, 1], f32, tag="sm")
                        nc.vector.tensor_reduce(sm[:], ex[:],
                                                mybir.AxisListType.X, ALU.add)
                        ls = smp.tile([128, 1], f32, tag="ls")
                        nc.scalar.activation(ls[:], sm[:], ACTF.Ln)
                        res = smp.tile([128, CLS], bf16, tag="res")
                        nc.vector.tensor_scalar(res[:], sh[:], ls[:, 0:1], None,
                                                ALU.subtract)
                        rows = NPC - t * 128 if t == TPC - 1 else 128
                        nc.sync.dma_start(out_h[t * 128:t * 128 + rows, :],
                                          res[0:rows, :])

    nc.compile()
    return nc


def _make_in_maps(inputs, gidx_all, dstv_all, degp_all, xsh_all):
    w1l = np.asarray(np.asarray(inputs["W1_l"], np.float32),
                     ml_dtypes.bfloat16)
    w1r = np.asarray(np.asarray(inputs["W1_r"], np.float32),
                     ml_dtypes.bfloat16)
    w2lf = np.asarray(inputs["W2_l"], np.float32)
    w2rf = np.asarray(inputs["W2_r"], np.float32)
    w2l = np.ascontiguousarray(
        np.concatenate([w2lf[:128, :], w2lf[128:, :]], axis=1)).astype(
            ml_dtypes.bfloat16)
    w2r = np.ascontiguousarray(
        np.concatenate([w2rf[:128, :], w2rf[128:, :]], axis=1)).astype(
            ml_dtypes.bfloat16)
    b1c = np.ascontiguousarray(
        np.asarray(inputs["b1"], np.float32).reshape(2, 128).T)
    b2r = np.ascontiguousarray(
        np.asarray(inputs["b2"], np.float32).reshape(1, CLS))
    in_maps = []
    for c in range(CORES):
        in_maps.append({
            "xsh": xsh_all[c],
            "gidx": gidx_all[c],
            "dstv": dstv_all[c],
            "degp": degp_all[c],
            "w1l": w1l, "w1r": w1r, "w2l": w2l, "w2r": w2r,
            "b1c": b1c, "b2r": b2r,
        })
    return in_maps


def _run(inputs, trace=False):
    x = np.asarray(inputs["x"], np.float32)
    edge_index = np.asarray(inputs["edge_index"])
    sched, gidx_all, dstv_all, degp_all, xsh_all = _host_prep(x, edge_index)
    nc = _build(sched)
    in_maps = _make_in_maps(inputs, gidx_all, dstv_all, degp_all, xsh_all)
    res = run_bass_kernel_spmd(nc, in_maps, core_ids=list(range(CORES)),
                               trace=trace)
    out = np.concatenate(
        [np.asarray(r["out"], np.float32) for r in res.results], axis=0)
    return out, res


def kernel(**inputs):
    out, _ = _run(inputs, trace=False)
    return out


# revision 10
# speedup vs baseline: 6.3755x; 1.2340x over previous
"""GraphSAGE 2-layer forward on 8 TRN2 NeuronCores — scatter-add variant.

Strategy (graph/data parallel per sharding hint):
- Nodes dst-sharded across 8 cores (6250 nodes/core, 49 tiles of 128).
- x is sharded host-side: each core receives only its own 6250-row bf16
  shard; the full bf16 feature table is assembled ON DEVICE via AllGather.
- Host sorts edges by dst and splits them by src<32768 (dma_gather idx is
  int16). Per layer the edge stream is processed in flat groups of 6144
  slots: gpsimd.dma_gather pulls x[src]/p[src] rows into SBUF, DVE upcasts
  to f32, and gpsimd.dma_scatter_add segment-sums them into a zeroed DRAM
  accumulator indexed by local dst (pad slots scatter into a trash tile).
  This replaces the one-hot-matmul scatter of the earlier revision and
  shrinks the program ~3x (walrus recompiles the NEFF on every call under
  the axon redirect, so BIR size is wall-clock).
- L1 per dst tile: mean = agg * 1/deg; DMA-transpose mean and the own-x
  tile; dense W1_l/W1_r matmuls (bf16) with fused bias+relu into h1T.
- h kept transposed [hid, nodes] bf16 in SBUF; p = h @ W2_l computed
  row-major, AllGathered (bf16, 128-col padded rows) so every core can
  gather p[src].
- L2: same gather/scatter machinery on p; + h @ W2_r + b2; log_softmax
  along the free dim; bf16 out, upcast on host.
"""

import numpy as np
import ml_dtypes

import concourse.bacc as bacc
import concourse.bass as bass
import concourse.mybir as mybir
import concourse.tile as tile
from concourse.bass_utils import run_bass_kernel_spmd

N = 50000
F = 128
HID = 256
CLS = 47
CORES = 8
NPC = N // CORES           # 6250
TPC = (NPC + 127) // 128   # 49 tiles per core
SPLIT = 32768              # int16 index limit for dma_gather
TRASH = TPC * 128          # first trash row of the dram accumulator
AGGR = TPC * 128 + 128     # accumulator rows incl. trash tile

f32 = mybir.dt.float32
bf16 = mybir.dt.bfloat16
i16 = mybir.dt.int16
i8 = mybir.dt.int8
ALU = mybir.AluOpType
ACTF = mybir.ActivationFunctionType

# weight blob layout (bf16): w1l | w1r | w2l_pad | w2r_pad. Each region
# is padded to a 128-multiple column width so SBUF loads map affinely.
# Only core-invariant data may live here: the blob is reassembled on
# device from DIFFERENT cores' shards by the AllGather.
_W1N = F * HID                      # 32768 = 256 blob rows
_W2N = 128 * 128                    # w2l padded [128, 94->128]
_WROWS = (2 * _W1N + 2 * _W2N) // 128            # 768 rows, /8 = 96
_WSH = _WROWS // 8                  # rows per core


def _rup128(v):
    return (int(v) + 127) // 128 * 128


def _host_prep(x, edge_index):
    src = np.asarray(edge_index[0], np.int64)
    dst = np.asarray(edge_index[1], np.int64)
    deg = np.bincount(dst, minlength=N).astype(np.float32)

    order = np.argsort(dst, kind="stable")
    src_s = src[order]
    dst_s = dst[order]
    bounds = np.searchsorted(dst_s, np.arange(0, N + 1, NPC))

    # dma_scatter_add does NOT serialize same-row RMW: duplicate dst
    # indices within one scatter lose updates (verified on HW). Split the
    # edge stream into rounds — round r holds the r-th edge of each dst —
    # so indices are unique per scatter; rounds are serialized by barriers.
    per_core = []
    for c in range(CORES):
        sl = slice(bounds[c], bounds[c + 1])
        sc = src_s[sl]
        dl = dst_s[sl] - c * NPC
        rank = np.arange(len(dl)) - np.searchsorted(dl, dl)
        seg = (sc >= SPLIT).astype(np.int64)
        o2 = np.lexsort((dl, seg, rank))
        per_core.append((sc[o2], dl[o2], seg[o2], rank[o2]))

    R = 1 + max(int(pc[3].max()) for pc in per_core)
    cnt = np.zeros((CORES, R, 2), np.int64)
    for c in range(CORES):
        np.add.at(cnt[c], (per_core[c][3], per_core[c][2]), 1)
    Ns = ((cnt.max(axis=0) + 127) // 128) * 128          # [R, 2]

    rounds = []
    off = 0
    for r in range(R):
        rounds.append((off, int(Ns[r, 0]), int(Ns[r, 1])))
        off += int(Ns[r, 0] + Ns[r, 1])
    total = off
    cols = total // 16

    gidx_all, sidx_all, degp_all, xsh_all = [], [], [], []
    xscale = float(np.abs(x).max()) / 127.0
    xq = np.clip(np.round(x / xscale), -127, 127).astype(np.int8)
    trash = TRASH + (np.arange(total) % 128)
    for c in range(CORES):
        gstream = np.zeros(total, np.int64)   # pad gathers row 0
        sstream = trash.copy()                # pad scatters -> trash
        sc, dl, seg, rank = per_core[c]
        # per-(round, seg) contiguous slices of the lexsorted edge arrays
        csum = np.concatenate([[0], np.cumsum(cnt[c].reshape(-1))])
        for r in range(R):
            a = rounds[r][0]
            for s in (0, 1):
                i0, i1 = csum[r * 2 + s], csum[r * 2 + s + 1]
                n = i1 - i0
                if n == 0:
                    continue
                p0 = a if s == 0 else a + int(Ns[r, 0])
                gv = sc[i0:i1] - (SPLIT if s == 1 else 0)
                gstream[p0:p0 + n] = gv
                sstream[p0:p0 + n] = dl[i0:i1]
        gidx_all.append(np.ascontiguousarray(
            gstream.reshape(-1, 16).T.astype(np.int16)))
        sidx_all.append(np.ascontiguousarray(
            sstream.reshape(-1, 16).T.astype(np.int16)))
        dpc = np.ones(TPC * 128, np.float32)
        dpc[:NPC] = deg[c * NPC:(c + 1) * NPC]
        degp_all.append(np.ascontiguousarray(dpc.reshape(TPC, 128).T))
        xp = np.zeros((TPC * 128, F), np.int8)
        xp[:NPC] = xq[c * NPC:(c + 1) * NPC]
        xsh_all.append(xp)

    maxc = max((nl + nh) // 128 for (_, nl, nh) in rounds)
    sched = dict(cols=cols, rounds=rounds, maxc=maxc, xscale=xscale)
    return sched, gidx_all, sidx_all, degp_all, xsh_all


def _build(sched):
    rounds, cols, maxc = sched["rounds"], sched["cols"], sched["maxc"]
    xscale = sched["xscale"]

    nc = bacc.Bacc("TRN2", num_devices=CORES)
    xsh_h = nc.declare_dram_parameter("xsh", [TPC * 128, F], i8, False)
    gidx_h = nc.declare_dram_parameter("gidx", [16, cols], i16, False)
    sidx_h = nc.declare_dram_parameter("sidx", [16, cols], i16, False)
    wsh_h = nc.declare_dram_parameter("wsh", [_WSH, 128], bf16, False)
    degp_h = nc.declare_dram_parameter("degp", [128, TPC], bf16, False)
    b1_h = nc.declare_dram_parameter("b1c", [128, 2], f32, False)
    b2_h = nc.declare_dram_parameter("b2r", [1, CLS], f32, False)
    out_h = nc.declare_dram_parameter("out", [NPC, CLS], bf16, True)

    x_loc = nc.dram_tensor("x_loc", [TPC * 128, F], bf16)
    x_full = nc.dram_tensor("x_full", [N, F], bf16, addr_space="Shared")
    w_loc = nc.dram_tensor("w_loc", [_WSH, 128], bf16)
    w_full = nc.dram_tensor("w_full", [_WROWS, 128], bf16,
                            addr_space="Shared")
    p_loc = nc.dram_tensor("p_loc", [NPC, 128], bf16)
    p_full = nc.dram_tensor("p_full", [N, 128], bf16, addr_space="Shared")
    agg1 = nc.dram_tensor("agg1", [AGGR, F], f32)
    agg2 = nc.dram_tensor("agg2", [AGGR, 128], f32)

    with tile.TileContext(nc) as tc:
        with (
            tc.tile_pool(name="const", bufs=1) as cp,
            tc.tile_pool(name="msg", bufs=2) as msgp,
            tc.tile_pool(name="msgf", bufs=2) as msgfp,
            tc.tile_pool(name="sb", bufs=3) as sbp,
            tc.tile_pool(name="small", bufs=4) as smp,
        ):
            # ---- dequantize the int8 x shard to bf16, then AllGather ----
            # (collectives cannot read IO tensors: stage via internal dram)
            for r0 in range(0, TPC * 128, 1024):
                rr = min(1024, TPC * 128 - r0)
                a = rr // 128
                xi = sbp.tile([128, 1024], i8, tag="xq")
                nc.sync.dma_start(
                    xi[:, 0:rr].rearrange("p (a f) -> p a f", f=F),
                    xsh_h[r0:r0 + rr, :].rearrange("(a b) f -> b a f",
                                                   b=128))
                xb = sbp.tile([128, 1024], bf16, tag="xb")
                nc.scalar.activation(xb[:, 0:rr], xi[:, 0:rr], ACTF.Copy,
                                     bias=0.0, scale=float(xscale))
                nc.sync.dma_start(
                    x_loc[r0:r0 + rr, :].rearrange("(a b) f -> b a f",
                                                   b=128),
                    xb[:, 0:rr].rearrange("p (a f) -> p a f", f=F))
            nc.gpsimd.collective_compute(
                "AllGather", ALU.bypass,
                replica_groups=[list(range(CORES))],
                ins=[x_loc[0:NPC, :].opt()], outs=[x_full.ap().opt()])
            # ---- weights travel sharded too: AllGather the blob ----
            nc.sync.dma_start(w_loc.ap(), wsh_h[:, :])
            nc.gpsimd.collective_compute(
                "AllGather", ALU.bypass,
                replica_groups=[list(range(CORES))],
                ins=[w_loc.ap().opt()], outs=[w_full.ap().opt()])

            # ---- persistent tiles ----
            # dma_gather/scatter want the idx block replicated across the
            # 8 Q7 cores (16 partitions each) — replicate on device.
            gidx_sb = cp.tile([128, cols], i16, tag="gidx")
            sidx_sb = cp.tile([128, cols], i16, tag="sidx")
            for idx_sb, idx_h in ((gidx_sb, gidx_h), (sidx_sb, sidx_h)):
                nc.sync.dma_start(idx_sb[0:16, :], idx_h[:, :])
                nc.sync.dma_start(idx_sb[16:32, :], idx_sb[0:16, :])
                nc.sync.dma_start(idx_sb[32:64, :], idx_sb[0:32, :])
                nc.sync.dma_start(idx_sb[64:128, :], idx_sb[0:64, :])

            b1_sb = cp.tile([128, 2], f32, tag="b1")
            nc.sync.dma_start(b1_sb[:], b1_h[:, :])
            b2_sb = cp.tile([1, CLS], f32, tag="b2")
            nc.sync.dma_start(b2_sb[:], b2_h[:, :])

            # weight loads read the AllGathered blob — barrier first
            tc.strict_bb_all_engine_barrier()
            w1l_sb = cp.tile([F, HID], bf16, tag="w1l")
            w1r_sb = cp.tile([F, HID], bf16, tag="w1r")
            w2l_sb = cp.tile([128, 128], bf16, tag="w2l")
            w2r_sb = cp.tile([128, 128], bf16, tag="w2r")
            o = 0
            for wt, nel in ((w1l_sb, _W1N), (w1r_sb, _W1N),
                            (w2l_sb, _W2N), (w2r_sb, _W2N)):
                rows = nel // 128
                a = rows // 128
                if a > 1:
                    nc.sync.dma_start(
                        wt[:].rearrange("p (a f) -> p a f", f=128),
                        w_full[o:o + rows, :].rearrange("(p a) f -> p a f",
                                                        a=a))
                else:
                    nc.sync.dma_start(wt[:], w_full[o:o + rows, :])
                o += rows
            degb_sb = cp.tile([128, TPC], bf16, tag="degb")
            nc.sync.dma_start(degb_sb[:], degp_h[:, :])
            deg_sb = cp.tile([128, TPC], f32, tag="deg")
            nc.vector.tensor_copy(deg_sb[:], degb_sb[:])

            invc_sb = cp.tile([128, TPC], f32, tag="invc")
            nc.vector.tensor_scalar(invc_sb[:], deg_sb[:], 1.0, None, ALU.max)
            inv_sb = cp.tile([128, TPC], f32, tag="inv")
            nc.vector.reciprocal_approx_fast(inv_sb[:], invc_sb[:])

            ones_sb = cp.tile([1, 128], f32, tag="ones")
            nc.vector.memset(ones_sb[:], 1.0)

            # ---- zero the dram accumulators (written once per layer) ----
            zsb = cp.tile([128, 1024], f32, tag="zero")
            nc.vector.memset(zsb[:], 0.0)
            for agg in (agg1, agg2):
                for r0 in range(0, AGGR, 1024):
                    rr = min(1024, AGGR - r0)
                    nc.sync.dma_start(
                        agg[r0:r0 + rr, :].rearrange("r f -> f r"),
                        zsb[:, 0:rr])

            h1T0 = cp.tile([128, TPC * 128], bf16, tag="h1a")
            h1T1 = cp.tile([128, TPC * 128], bf16, tag="h1b")

            # zero-fills and the x AllGather staging must land before any
            # scatter-add / gather touches the dram tensors.
            tc.strict_bb_all_engine_barrier()

            def sweep(table_full, agg_dram):
                """gather rows -> upcast f32 -> scatter-add into agg_dram.

                One scatter per round (unique dst indices); a barrier
                before each scatter serializes the same-row RMW between
                rounds while letting the next round's gather overlap."""
                for (a, n_lo, n_hi) in rounds:
                    C = (n_lo + n_hi) // 128
                    msg = msgp.tile([128, maxc * F], bf16, tag="msg")
                    msg3 = msg[:].rearrange("p (c e) -> p c e", e=F)
                    if n_lo:
                        nc.gpsimd.dma_gather(
                            msg3[:, 0:n_lo // 128, :], table_full[0:SPLIT, :],
                            gidx_sb[:, a // 16:(a + n_lo) // 16],
                            n_lo, n_lo, F, single_packet=False)
                    if n_hi:
                        nc.gpsimd.dma_gather(
                            msg3[:, n_lo // 128:C, :], table_full[SPLIT:N, :],
                            gidx_sb[:, (a + n_lo) // 16:(a + n_lo + n_hi) // 16],
                            n_hi, n_hi, F, single_packet=False)
                    msgf = msgfp.tile([128, maxc * F], f32, tag="msgf")
                    nc.vector.tensor_copy(msgf[:, 0:C * F], msg[:, 0:C * F])
                    msgf3 = msgf[:].rearrange("p (c e) -> p c e", e=F)
                    tc.strict_bb_all_engine_barrier()
                    nc.gpsimd.dma_scatter_add(
                        agg_dram[:, :], msgf3[:, 0:C, :],
                        sidx_sb[:, a // 16:(a + n_lo + n_hi) // 16],
                        n_lo + n_hi, n_lo + n_hi, F, single_packet=False)

            # =============== Layer 1 ===============
            # scatter-add writes to dram are not visible to the tile
            # dependency tracker — hard barrier before the agg reads.
            sweep(x_full, agg1)
            tc.strict_bb_all_engine_barrier()
            with tc.tile_pool(name="zp", bufs=2, space="PSUM") as zpp:
                for t in range(TPC):
                    agg_sb = sbp.tile([128, F], f32, tag="agg")
                    nc.sync.dma_start(agg_sb[:], agg1[t * 128:(t + 1) * 128, :])
                    mean = sbp.tile([128, F], bf16, tag="mean")
                    nc.vector.tensor_scalar(mean[:], agg_sb[:],
                                            inv_sb[:, t:t + 1], None, ALU.mult)
                    meanT = sbp.tile([128, 128], bf16, tag="meanT")
                    nc.sync.dma_start_transpose(meanT[:], mean[:])
                    xoT = sbp.tile([128, 128], bf16, tag="xoT")
                    nc.sync.dma_start_transpose(
                        xoT[:], x_loc[t * 128:(t + 1) * 128, :])
                    z_ps = zpp.tile([128, 256], f32, tag="z")
                    for h, h1T in ((0, h1T0), (1, h1T1)):
                        zs = z_ps[:, h * 128:(h + 1) * 128]
                        nc.tensor.matmul(zs, w1l_sb[:, h * 128:(h + 1) * 128],
                                         meanT[:], start=True, stop=False)
                        nc.tensor.matmul(zs, w1r_sb[:, h * 128:(h + 1) * 128],
                                         xoT[:], start=False, stop=True)
                        nc.scalar.activation(h1T[:, t * 128:(t + 1) * 128],
                                             zs, ACTF.Relu,
                                             bias=b1_sb[:, h:h + 1],
                                             scale=1.0)

            # =============== p = h @ W2_l, AllGather ===============
            with tc.tile_pool(name="pp", bufs=2, space="PSUM") as ppp:
                for t in range(TPC):
                    ts = slice(t * 128, (t + 1) * 128)
                    pp_ps = ppp.tile([128, 64], f32, tag="pp")
                    nc.tensor.matmul(pp_ps[:, 0:CLS], h1T0[:, ts],
                                     w2l_sb[:, 0:CLS], start=True, stop=False)
                    nc.tensor.matmul(pp_ps[:, 0:CLS], h1T1[:, ts],
                                     w2l_sb[:, CLS:2 * CLS], start=False,
                                     stop=True)
                    psb = sbp.tile([128, 128], bf16, tag="psb")
                    nc.vector.memset(psb[:, CLS:128], 0.0)
                    nc.scalar.activation(psb[:, 0:CLS], pp_ps[:, 0:CLS],
                                         ACTF.Copy)
                    rows = NPC - t * 128 if t == TPC - 1 else 128
                    nc.sync.dma_start(p_loc[t * 128:t * 128 + rows, :],
                                      psb[0:rows, :])

                nc.gpsimd.collective_compute(
                    "AllGather", ALU.bypass,
                    replica_groups=[list(range(CORES))],
                    ins=[p_loc.ap().opt()], outs=[p_full.ap().opt()])

                # b2 broadcast across partitions via rank-1 matmul
                b2_ps = ppp.tile([128, 64], f32, tag="pp")
                nc.tensor.matmul(b2_ps[:, 0:CLS], ones_sb[0:1, :],
                                 b2_sb[0:1, :], start=True, stop=True)
                b2bc = cp.tile([128, CLS], f32, tag="b2bc")
                nc.scalar.activation(b2bc[:], b2_ps[:, 0:CLS], ACTF.Copy)

            # =============== Layer 2 ===============
            sweep(p_full, agg2)
            tc.strict_bb_all_engine_barrier()
            with tc.tile_pool(name="op", bufs=2, space="PSUM") as opp:
                for t in range(TPC):
                    agg_sb = smp.tile([128, CLS], f32, tag="agg2")
                    nc.sync.dma_start(agg_sb[:],
                                      agg2[t * 128:(t + 1) * 128, 0:CLS])
                    s_sb = smp.tile([128, CLS], f32, tag="s")
                    nc.vector.tensor_scalar(s_sb[:], agg_sb[:],
                                            inv_sb[:, t:t + 1], None, ALU.mult)
                    ts = slice(t * 128, (t + 1) * 128)
                    o_ps = opp.tile([128, 64], f32, tag="op")
                    nc.tensor.matmul(o_ps[:, 0:CLS], h1T0[:, ts],
                                     w2r_sb[:, 0:CLS], start=True, stop=False)
                    nc.tensor.matmul(o_ps[:, 0:CLS], h1T1[:, ts],
                                     w2r_sb[:, CLS:2 * CLS], start=False,
                                     stop=True)
                    lg = smp.tile([128, CLS], f32, tag="lg")
                    nc.vector.tensor_tensor(lg[:], o_ps[:, 0:CLS], s_sb[:],
                                            ALU.add)
                    lg2 = smp.tile([128, CLS], f32, tag="lg2")
                    nc.vector.tensor_tensor(lg2[:], lg[:], b2bc[:], ALU.add)
                    mx = smp.tile([128, 1], f32, tag="mx")
                    nc.vector.tensor_reduce(mx[:], lg2[:],
                                            mybir.AxisListType.X, ALU.max)
                    sh = smp.tile([128, CLS], f32, tag="sh")
                    nc.vector.tensor_scalar(sh[:], lg2[:], mx[:, 0:1], None,
                                            ALU.subtract)
                    ex = smp.tile([128, CLS], f32, tag="ex")
                    nc.scalar.activation(ex[:], sh[:], ACTF.Exp)
                    sm = smp.tile([128, 1], f32, tag="sm")
                    nc.vector.tensor_reduce(sm[:], ex[:],
                                            mybir.AxisListType.X, ALU.add)
                    ls = smp.tile([128, 1], f32, tag="ls")
                    nc.scalar.activation(ls[:], sm[:], ACTF.Ln)
                    res = smp.tile([128, CLS], bf16, tag="res")
                    nc.vector.tensor_scalar(res[:], sh[:], ls[:, 0:1], None,
                                            ALU.subtract)
                    rows = NPC - t * 128 if t == TPC - 1 else 128
                    nc.sync.dma_start(out_h[t * 128:t * 128 + rows, :],
                                      res[0:rows, :])

    nc.compile()
    return nc


def _make_in_maps(inputs, gidx_all, sidx_all, degp_all, xsh_all):
    w1l = np.asarray(np.asarray(inputs["W1_l"], np.float32),
                     ml_dtypes.bfloat16)
    w1r = np.asarray(np.asarray(inputs["W1_r"], np.float32),
                     ml_dtypes.bfloat16)
    w2lf = np.asarray(inputs["W2_l"], np.float32)
    w2rf = np.asarray(inputs["W2_r"], np.float32)
    w2l = np.ascontiguousarray(
        np.concatenate([w2lf[:128, :], w2lf[128:, :]], axis=1)).astype(
            ml_dtypes.bfloat16)
    w2r = np.ascontiguousarray(
        np.concatenate([w2rf[:128, :], w2rf[128:, :]], axis=1)).astype(
            ml_dtypes.bfloat16)
    w2lp = np.zeros((128, 128), ml_dtypes.bfloat16)
    w2lp[:, 0:2 * CLS] = w2l
    w2rp = np.zeros((128, 128), ml_dtypes.bfloat16)
    w2rp[:, 0:2 * CLS] = w2r
    blob = np.zeros(_WROWS * 128, ml_dtypes.bfloat16)
    o = 0
    for a in (w1l, w1r, w2lp, w2rp):
        blob[o:o + a.size] = a.reshape(-1)
        o += a.size
    blob2 = blob.reshape(_WROWS, 128)
    b1c = np.ascontiguousarray(
        np.asarray(inputs["b1"], np.float32).reshape(2, 128).T)
    b2r = np.ascontiguousarray(
        np.asarray(inputs["b2"], np.float32).reshape(1, CLS))
    in_maps = []
    for c in range(CORES):
        in_maps.append({
            "xsh": xsh_all[c],
            "gidx": gidx_all[c],
            "sidx": sidx_all[c],
            "wsh": np.ascontiguousarray(blob2[c * _WSH:(c + 1) * _WSH]),
            "degp": degp_all[c].astype(ml_dtypes.bfloat16),
            "b1c": b1c, "b2r": b2r,
        })
    return in_maps


def _run(inputs, trace=False):
    x = np.asarray(inputs["x"], np.float32)
    edge_index = np.asarray(inputs["edge_index"])
    sched, gidx_all, sidx_all, degp_all, xsh_all = _host_prep(x, edge_index)
    nc = _build(sched)
    in_maps = _make_in_maps(inputs, gidx_all, sidx_all, degp_all, xsh_all)
    res = run_bass_kernel_spmd(nc, in_maps, core_ids=list(range(CORES)),
                               trace=trace)
    out = np.concatenate(
        [np.asarray(r["out"], np.float32) for r in res.results], axis=0)
    return out, res


def kernel(**inputs):
    out, _ = _run(inputs, trace=False)
    return out


# revision 11
# speedup vs baseline: 6.7962x; 1.0660x over previous
"""GraphSAGE 2-layer forward on 8 TRN2 NeuronCores — scatter-add variant.

Strategy (graph/data parallel per sharding hint):
- Nodes dst-sharded across 8 cores (6250 nodes/core, 49 tiles of 128).
- x is sharded host-side: each core receives only its own 6250-row bf16
  shard; the full bf16 feature table is assembled ON DEVICE via AllGather.
- Host sorts edges by dst and splits them by src<32768 (dma_gather idx is
  int16), then slices the stream into ROUNDS: round r holds the r-th edge
  of each dst node, so dst indices are unique within a round. Per round:
  gpsimd.dma_gather pulls x[src]/p[src] rows into SBUF, DVE upcasts to
  f32, and gpsimd.dma_scatter_add segment-sums them into a zeroed DRAM
  accumulator indexed by local dst (pad slots scatter into a trash tile).
  Rounds are serialized by barriers because dma_scatter_add does not
  serialize same-row read-modify-write (verified on HW: duplicate indices
  lose updates). This replaces the one-hot-matmul scatter of the earlier
  revision and shrinks the program ~2x (walrus recompiles the NEFF on
  every call under the axon redirect, so BIR size is wall-clock).
- x ships as int8 (device dequant, scale baked into the NEFF) and the
  dense weights ship sharded 8 ways and are AllGathered on device; with
  the bf16 output this cuts host->device traffic ~15x vs the baseline.
- L1 per dst tile: mean = agg * 1/deg; DMA-transpose mean and the own-x
  tile; dense W1_l/W1_r matmuls (bf16) with fused bias+relu into h1T.
- h kept transposed [hid, nodes] bf16 in SBUF; p = h @ W2_l computed
  row-major, AllGathered (bf16, 128-col padded rows) so every core can
  gather p[src].
- L2: same gather/scatter machinery on p; + h @ W2_r + b2; log_softmax
  along the free dim; bf16 out, upcast on host.
"""

import numpy as np
import ml_dtypes

import concourse.bacc as bacc
import concourse.bass as bass
import concourse.mybir as mybir
import concourse.tile as tile
from concourse.bass_utils import run_bass_kernel_spmd

N = 50000
F = 128
HID = 256
CLS = 47
CORES = 8
NPC = N // CORES           # 6250
TPC = (NPC + 127) // 128   # 49 tiles per core
SPLIT = 32768              # int16 index limit for dma_gather
TRASH = TPC * 128          # first trash row of the dram accumulator
AGGR = TPC * 128 + 128     # accumulator rows incl. trash tile

f32 = mybir.dt.float32
bf16 = mybir.dt.bfloat16
i16 = mybir.dt.int16
i8 = mybir.dt.int8
ALU = mybir.AluOpType
ACTF = mybir.ActivationFunctionType

# weight blob layout (bf16): w1l | w1r | w2l_pad | w2r_pad. Each region
# is padded to a 128-multiple column width so SBUF loads map affinely.
# Only core-invariant data may live here: the blob is reassembled on
# device from DIFFERENT cores' shards by the AllGather.
_W1N = F * HID                      # 32768 = 256 blob rows
_W2N = 128 * 128                    # w2l padded [128, 94->128]
_WROWS = (2 * _W1N + 2 * _W2N) // 128            # 768 rows, /8 = 96
_WSH = _WROWS // 8                  # rows per core


def _rup128(v):
    return (int(v) + 127) // 128 * 128


def _host_prep(x, edge_index):
    src = np.asarray(edge_index[0], np.int64)
    dst = np.asarray(edge_index[1], np.int64)
    deg = np.bincount(dst, minlength=N).astype(np.float32)

    order = np.argsort(dst, kind="stable")
    src_s = src[order]
    dst_s = dst[order]
    bounds = np.searchsorted(dst_s, np.arange(0, N + 1, NPC))

    # dma_scatter_add does NOT serialize same-row RMW: duplicate dst
    # indices within one scatter lose updates (verified on HW). Split the
    # edge stream into rounds — round r holds the r-th edge of each dst —
    # so indices are unique per scatter; rounds are serialized by barriers.
    per_core = []
    for c in range(CORES):
        sl = slice(bounds[c], bounds[c + 1])
        sc = src_s[sl]
        dl = dst_s[sl] - c * NPC
        rank = np.arange(len(dl)) - np.searchsorted(dl, dl)
        seg = (sc >= SPLIT).astype(np.int64)
        o2 = np.lexsort((dl, seg, rank))
        per_core.append((sc[o2], dl[o2], seg[o2], rank[o2]))

    R = 1 + max(int(pc[3].max()) for pc in per_core)
    cnt = np.zeros((CORES, R, 2), np.int64)
    for c in range(CORES):
        np.add.at(cnt[c], (per_core[c][3], per_core[c][2]), 1)
    Ns = ((cnt.max(axis=0) + 127) // 128) * 128          # [R, 2]

    rounds = []
    off = 0
    for r in range(R):
        rounds.append((off, int(Ns[r, 0]), int(Ns[r, 1])))
        off += int(Ns[r, 0] + Ns[r, 1])
    total = off
    cols = total // 16

    gidx_all, sidx_all, degp_all, xsh_all = [], [], [], []
    xscale = float(np.abs(x).max()) / 127.0
    xq = np.clip(np.round(x / xscale), -127, 127).astype(np.int8)
    trash = TRASH + (np.arange(total) % 128)
    for c in range(CORES):
        gstream = np.zeros(total, np.int64)   # pad gathers row 0
        sstream = trash.copy()                # pad scatters -> trash
        sc, dl, seg, rank = per_core[c]
        # per-(round, seg) contiguous slices of the lexsorted edge arrays
        csum = np.concatenate([[0], np.cumsum(cnt[c].reshape(-1))])
        for r in range(R):
            a = rounds[r][0]
            for s in (0, 1):
                i0, i1 = csum[r * 2 + s], csum[r * 2 + s + 1]
                n = i1 - i0
                if n == 0:
                    continue
                p0 = a if s == 0 else a + int(Ns[r, 0])
                gv = sc[i0:i1] - (SPLIT if s == 1 else 0)
                gstream[p0:p0 + n] = gv
                sstream[p0:p0 + n] = dl[i0:i1]
        gidx_all.append(np.ascontiguousarray(
            gstream.reshape(-1, 16).T.astype(np.int16)))
        sidx_all.append(np.ascontiguousarray(
            sstream.reshape(-1, 16).T.astype(np.int16)))
        dpc = np.ones(TPC * 128, np.float32)
        dpc[:NPC] = deg[c * NPC:(c + 1) * NPC]
        degp_all.append(np.ascontiguousarray(dpc.reshape(TPC, 128).T))
        xp = np.zeros((TPC * 128, F), np.int8)
        xp[:NPC] = xq[c * NPC:(c + 1) * NPC]
        xsh_all.append(xp)

    maxc = max((nl + nh) // 128 for (_, nl, nh) in rounds)
    sched = dict(cols=cols, rounds=rounds, maxc=maxc, xscale=xscale)
    return sched, gidx_all, sidx_all, degp_all, xsh_all


def _build(sched):
    rounds, cols, maxc = sched["rounds"], sched["cols"], sched["maxc"]
    xscale = sched["xscale"]

    nc = bacc.Bacc("TRN2", num_devices=CORES)
    xsh_h = nc.declare_dram_parameter("xsh", [TPC * 128, F], i8, False)
    gidx_h = nc.declare_dram_parameter("gidx", [16, cols], i16, False)
    sidx_h = nc.declare_dram_parameter("sidx", [16, cols], i16, False)
    wsh_h = nc.declare_dram_parameter("wsh", [_WSH, 128], bf16, False)
    degp_h = nc.declare_dram_parameter("degp", [128, TPC], bf16, False)
    b1_h = nc.declare_dram_parameter("b1c", [128, 2], f32, False)
    b2_h = nc.declare_dram_parameter("b2r", [1, CLS], f32, False)
    out_h = nc.declare_dram_parameter("out", [NPC, CLS], bf16, True)

    x_loc = nc.dram_tensor("x_loc", [TPC * 128, F], bf16)
    x_full = nc.dram_tensor("x_full", [N, F], bf16, addr_space="Shared")
    w_loc = nc.dram_tensor("w_loc", [_WSH, 128], bf16)
    w_full = nc.dram_tensor("w_full", [_WROWS, 128], bf16,
                            addr_space="Shared")
    p_loc = nc.dram_tensor("p_loc", [NPC, 128], bf16)
    p_full = nc.dram_tensor("p_full", [N, 128], bf16, addr_space="Shared")
    agg1 = nc.dram_tensor("agg1", [AGGR, F], f32)
    agg2 = nc.dram_tensor("agg2", [AGGR, 128], f32)

    with tile.TileContext(nc) as tc:
        with (
            tc.tile_pool(name="const", bufs=1) as cp,
            tc.tile_pool(name="msg", bufs=2) as msgp,
            tc.tile_pool(name="msgf", bufs=2) as msgfp,
            tc.tile_pool(name="sb", bufs=3) as sbp,
            tc.tile_pool(name="small", bufs=4) as smp,
        ):
            # ---- dequantize the int8 x shard to bf16, then AllGather ----
            # (collectives cannot read IO tensors: stage via internal dram)
            for r0 in range(0, TPC * 128, 1024):
                rr = min(1024, TPC * 128 - r0)
                a = rr // 128
                xi = sbp.tile([128, 1024], i8, tag="xq")
                nc.sync.dma_start(
                    xi[:, 0:rr].rearrange("p (a f) -> p a f", f=F),
                    xsh_h[r0:r0 + rr, :].rearrange("(a b) f -> b a f",
                                                   b=128))
                xb = sbp.tile([128, 1024], bf16, tag="xb")
                nc.scalar.activation(xb[:, 0:rr], xi[:, 0:rr], ACTF.Copy,
                                     bias=0.0, scale=float(xscale))
                nc.sync.dma_start(
                    x_loc[r0:r0 + rr, :].rearrange("(a b) f -> b a f",
                                                   b=128),
                    xb[:, 0:rr].rearrange("p (a f) -> p a f", f=F))
            nc.gpsimd.collective_compute(
                "AllGather", ALU.bypass,
                replica_groups=[list(range(CORES))],
                ins=[x_loc[0:NPC, :].opt()], outs=[x_full.ap().opt()])
            # ---- weights travel sharded too: AllGather the blob ----
            nc.sync.dma_start(w_loc.ap(), wsh_h[:, :])
            nc.gpsimd.collective_compute(
                "AllGather", ALU.bypass,
                replica_groups=[list(range(CORES))],
                ins=[w_loc.ap().opt()], outs=[w_full.ap().opt()])

            # ---- persistent tiles ----
            # dma_gather/scatter want the idx block replicated across the
            # 8 Q7 cores (16 partitions each) — replicate on device.
            gidx_sb = cp.tile([128, cols], i16, tag="gidx")
            sidx_sb = cp.tile([128, cols], i16, tag="sidx")
            for idx_sb, idx_h in ((gidx_sb, gidx_h), (sidx_sb, sidx_h)):
                nc.sync.dma_start(idx_sb[0:16, :], idx_h[:, :])
                nc.sync.dma_start(idx_sb[16:32, :], idx_sb[0:16, :])
                nc.sync.dma_start(idx_sb[32:64, :], idx_sb[0:32, :])
                nc.sync.dma_start(idx_sb[64:128, :], idx_sb[0:64, :])

            b1_sb = cp.tile([128, 2], f32, tag="b1")
            nc.sync.dma_start(b1_sb[:], b1_h[:, :])
            b2_sb = cp.tile([1, CLS], f32, tag="b2")
            nc.sync.dma_start(b2_sb[:], b2_h[:, :])

            # weight loads read the AllGathered blob — barrier first
            tc.strict_bb_all_engine_barrier()
            w1l_sb = cp.tile([F, HID], bf16, tag="w1l")
            w1r_sb = cp.tile([F, HID], bf16, tag="w1r")
            w2l_sb = cp.tile([128, 128], bf16, tag="w2l")
            w2r_sb = cp.tile([128, 128], bf16, tag="w2r")
            o = 0
            for wt, nel in ((w1l_sb, _W1N), (w1r_sb, _W1N),
                            (w2l_sb, _W2N), (w2r_sb, _W2N)):
                rows = nel // 128
                a = rows // 128
                if a > 1:
                    nc.sync.dma_start(
                        wt[:].rearrange("p (a f) -> p a f", f=128),
                        w_full[o:o + rows, :].rearrange("(p a) f -> p a f",
                                                        a=a))
                else:
                    nc.sync.dma_start(wt[:], w_full[o:o + rows, :])
                o += rows
            degb_sb = cp.tile([128, TPC], bf16, tag="degb")
            nc.sync.dma_start(degb_sb[:], degp_h[:, :])
            deg_sb = cp.tile([128, TPC], f32, tag="deg")
            nc.vector.tensor_copy(deg_sb[:], degb_sb[:])

            invc_sb = cp.tile([128, TPC], f32, tag="invc")
            nc.vector.tensor_scalar(invc_sb[:], deg_sb[:], 1.0, None, ALU.max)
            inv_sb = cp.tile([128, TPC], f32, tag="inv")
            nc.vector.reciprocal_approx_fast(inv_sb[:], invc_sb[:])

            ones_sb = cp.tile([1, 128], f32, tag="ones")
            nc.vector.memset(ones_sb[:], 1.0)

            # ---- zero the dram accumulators (written once per layer) ----
            zsb = cp.tile([128, 1024], f32, tag="zero")
            nc.vector.memset(zsb[:], 0.0)
            for agg in (agg1, agg2):
                for r0 in range(0, AGGR, 1024):
                    rr = min(1024, AGGR - r0)
                    nc.sync.dma_start(
                        agg[r0:r0 + rr, :].rearrange("r f -> f r"),
                        zsb[:, 0:rr])

            h1T0 = cp.tile([128, TPC * 128], bf16, tag="h1a")
            h1T1 = cp.tile([128, TPC * 128], bf16, tag="h1b")

            # zero-fills and the x AllGather staging must land before any
            # scatter-add / gather touches the dram tensors.
            tc.strict_bb_all_engine_barrier()

            def sweep(table_full, agg_dram):
                """gather rows -> upcast f32 -> scatter-add into agg_dram.

                One scatter per round (unique dst indices); a barrier
                before each scatter serializes the same-row RMW between
                rounds while letting the next round's gather overlap."""
                for (a, n_lo, n_hi) in rounds:
                    C = (n_lo + n_hi) // 128
                    msg = msgp.tile([128, maxc * F], bf16, tag="msg")
                    msg3 = msg[:].rearrange("p (c e) -> p c e", e=F)
                    if n_lo:
                        nc.gpsimd.dma_gather(
                            msg3[:, 0:n_lo // 128, :], table_full[0:SPLIT, :],
                            gidx_sb[:, a // 16:(a + n_lo) // 16],
                            n_lo, n_lo, F, single_packet=False)
                    if n_hi:
                        nc.gpsimd.dma_gather(
                            msg3[:, n_lo // 128:C, :], table_full[SPLIT:N, :],
                            gidx_sb[:, (a + n_lo) // 16:(a + n_lo + n_hi) // 16],
                            n_hi, n_hi, F, single_packet=False)
                    msgf = msgfp.tile([128, maxc * F], f32, tag="msgf")
                    nc.vector.tensor_copy(msgf[:, 0:C * F], msg[:, 0:C * F])
                    msgf3 = msgf[:].rearrange("p (c e) -> p c e", e=F)
                    tc.strict_bb_all_engine_barrier()
                    nc.gpsimd.dma_scatter_add(
                        agg_dram[:, :], msgf3[:, 0:C, :],
                        sidx_sb[:, a // 16:(a + n_lo + n_hi) // 16],
                        n_lo + n_hi, n_lo + n_hi, F, single_packet=False)

            # =============== Layer 1 ===============
            # scatter-add writes to dram are not visible to the tile
            # dependency tracker — hard barrier before the agg reads.
            sweep(x_full, agg1)
            tc.strict_bb_all_engine_barrier()
            with tc.tile_pool(name="zp", bufs=2, space="PSUM") as zpp:
                for t in range(TPC):
                    agg_sb = sbp.tile([128, F], f32, tag="agg")
                    nc.sync.dma_start(agg_sb[:], agg1[t * 128:(t + 1) * 128, :])
                    mean = sbp.tile([128, F], bf16, tag="mean")
                    nc.vector.tensor_scalar(mean[:], agg_sb[:],
                                            inv_sb[:, t:t + 1], None, ALU.mult)
                    meanT = sbp.tile([128, 128], bf16, tag="meanT")
                    nc.sync.dma_start_transpose(meanT[:], mean[:])
                    xoT = sbp.tile([128, 128], bf16, tag="xoT")
                    nc.sync.dma_start_transpose(
                        xoT[:], x_loc[t * 128:(t + 1) * 128, :])
                    z_ps = zpp.tile([128, 256], f32, tag="z")
                    for h, h1T in ((0, h1T0), (1, h1T1)):
                        zs = z_ps[:, h * 128:(h + 1) * 128]
                        nc.tensor.matmul(zs, w1l_sb[:, h * 128:(h + 1) * 128],
                                         meanT[:], start=True, stop=False)
                        nc.tensor.matmul(zs, w1r_sb[:, h * 128:(h + 1) * 128],
                                         xoT[:], start=False, stop=True)
                        nc.scalar.activation(h1T[:, t * 128:(t + 1) * 128],
                                             zs, ACTF.Relu,
                                             bias=b1_sb[:, h:h + 1],
                                             scale=1.0)

            # =============== p = h @ W2_l, AllGather ===============
            with tc.tile_pool(name="pp", bufs=2, space="PSUM") as ppp:
                for t in range(TPC):
                    ts = slice(t * 128, (t + 1) * 128)
                    pp_ps = ppp.tile([128, 64], f32, tag="pp")
                    nc.tensor.matmul(pp_ps[:, 0:CLS], h1T0[:, ts],
                                     w2l_sb[:, 0:CLS], start=True, stop=False)
                    nc.tensor.matmul(pp_ps[:, 0:CLS], h1T1[:, ts],
                                     w2l_sb[:, CLS:2 * CLS], start=False,
                                     stop=True)
                    psb = sbp.tile([128, 128], bf16, tag="psb")
                    nc.vector.memset(psb[:, CLS:128], 0.0)
                    nc.scalar.activation(psb[:, 0:CLS], pp_ps[:, 0:CLS],
                                         ACTF.Copy)
                    rows = NPC - t * 128 if t == TPC - 1 else 128
                    nc.sync.dma_start(p_loc[t * 128:t * 128 + rows, :],
                                      psb[0:rows, :])

                nc.gpsimd.collective_compute(
                    "AllGather", ALU.bypass,
                    replica_groups=[list(range(CORES))],
                    ins=[p_loc.ap().opt()], outs=[p_full.ap().opt()])

                # b2 broadcast across partitions via rank-1 matmul
                b2_ps = ppp.tile([128, 64], f32, tag="pp")
                nc.tensor.matmul(b2_ps[:, 0:CLS], ones_sb[0:1, :],
                                 b2_sb[0:1, :], start=True, stop=True)
                b2bc = cp.tile([128, CLS], f32, tag="b2bc")
                nc.scalar.activation(b2bc[:], b2_ps[:, 0:CLS], ACTF.Copy)

            # =============== Layer 2 ===============
            sweep(p_full, agg2)
            tc.strict_bb_all_engine_barrier()
            with tc.tile_pool(name="op", bufs=2, space="PSUM") as opp:
                for t in range(TPC):
                    agg_sb = smp.tile([128, CLS], f32, tag="agg2")
                    nc.sync.dma_start(agg_sb[:],
                                      agg2[t * 128:(t + 1) * 128, 0:CLS])
                    s_sb = smp.tile([128, CLS], f32, tag="s")
                    nc.vector.tensor_scalar(s_sb[:], agg_sb[:],
                                            inv_sb[:, t:t + 1], None, ALU.mult)
                    ts = slice(t * 128, (t + 1) * 128)
                    o_ps = opp.tile([128, 64], f32, tag="op")
                    nc.tensor.matmul(o_ps[:, 0:CLS], h1T0[:, ts],
                                     w2r_sb[:, 0:CLS], start=True, stop=False)
                    nc.tensor.matmul(o_ps[:, 0:CLS], h1T1[:, ts],
                                     w2r_sb[:, CLS:2 * CLS], start=False,
                                     stop=True)
                    lg = smp.tile([128, CLS], f32, tag="lg")
                    nc.vector.tensor_tensor(lg[:], o_ps[:, 0:CLS], s_sb[:],
                                            ALU.add)
                    lg2 = smp.tile([128, CLS], f32, tag="lg2")
                    nc.vector.tensor_tensor(lg2[:], lg[:], b2bc[:], ALU.add)
                    mx = smp.tile([128, 1], f32, tag="mx")
                    nc.vector.tensor_reduce(mx[:], lg2[:],
                                            mybir.AxisListType.X, ALU.max)
                    sh = smp.tile([128, CLS], f32, tag="sh")
                    nc.vector.tensor_scalar(sh[:], lg2[:], mx[:, 0:1], None,
                                            ALU.subtract)
                    ex = smp.tile([128, CLS], f32, tag="ex")
                    nc.scalar.activation(ex[:], sh[:], ACTF.Exp)
                    sm = smp.tile([128, 1], f32, tag="sm")
                    nc.vector.tensor_reduce(sm[:], ex[:],
                                            mybir.AxisListType.X, ALU.add)
                    ls = smp.tile([128, 1], f32, tag="ls")
                    nc.scalar.activation(ls[:], sm[:], ACTF.Ln)
                    res = smp.tile([128, CLS], bf16, tag="res")
                    nc.vector.tensor_scalar(res[:], sh[:], ls[:, 0:1], None,
                                            ALU.subtract)
                    rows = NPC - t * 128 if t == TPC - 1 else 128
                    nc.sync.dma_start(out_h[t * 128:t * 128 + rows, :],
                                      res[0:rows, :])

    nc.compile()
    return nc


def _make_in_maps(inputs, gidx_all, sidx_all, degp_all, xsh_all):
    w1l = np.asarray(np.asarray(inputs["W1_l"], np.float32),
                     ml_dtypes.bfloat16)
    w1r = np.asarray(np.asarray(inputs["W1_r"], np.float32),
                     ml_dtypes.bfloat16)
    w2lf = np.asarray(inputs["W2_l"], np.float32)
    w2rf = np.asarray(inputs["W2_r"], np.float32)
    w2l = np.ascontiguousarray(
        np.concatenate([w2lf[:128, :], w2lf[128:, :]], axis=1)).astype(
            ml_dtypes.bfloat16)
    w2r = np.ascontiguousarray(
        np.concatenate([w2rf[:128, :], w2rf[128:, :]], axis=1)).astype(
            ml_dtypes.bfloat16)
    w2lp = np.zeros((128, 128), ml_dtypes.bfloat16)
    w2lp[:, 0:2 * CLS] = w2l
    w2rp = np.zeros((128, 128), ml_dtypes.bfloat16)
    w2rp[:, 0:2 * CLS] = w2r
    blob = np.zeros(_WROWS * 128, ml_dtypes.bfloat16)
    o = 0
    for a in (w1l, w1r, w2lp, w2rp):
        blob[o:o + a.size] = a.reshape(-1)
        o += a.size
    blob2 = blob.reshape(_WROWS, 128)
    b1c = np.ascontiguousarray(
        np.asarray(inputs["b1"], np.float32).reshape(2, 128).T)
    b2r = np.ascontiguousarray(
        np.asarray(inputs["b2"], np.float32).reshape(1, CLS))
    in_maps = []
    for c in range(CORES):
        in_maps.append({
            "xsh": xsh_all[c],
            "gidx": gidx_all[c],
            "sidx": sidx_all[c],
            "wsh": np.ascontiguousarray(blob2[c * _WSH:(c + 1) * _WSH]),
            "degp": degp_all[c].astype(ml_dtypes.bfloat16),
            "b1c": b1c, "b2r": b2r,
        })
    return in_maps


def _run(inputs, trace=False):
    x = np.asarray(inputs["x"], np.float32)
    edge_index = np.asarray(inputs["edge_index"])
    sched, gidx_all, sidx_all, degp_all, xsh_all = _host_prep(x, edge_index)
    nc = _build(sched)
    in_maps = _make_in_maps(inputs, gidx_all, sidx_all, degp_all, xsh_all)
    res = run_bass_kernel_spmd(nc, in_maps, core_ids=list(range(CORES)),
                               trace=trace)
    out = np.concatenate(
        [np.asarray(r["out"], np.float32) for r in res.results], axis=0)
    return out, res


def kernel(**inputs):
    out, _ = _run(inputs, trace=False)
    return out
